# revision 17
# baseline (speedup 1.0000x reference)
"""Trainium2 Bass kernel for nn_CTCConsistencyLoss_7310034338203.

Data-parallel over batch: 8 cores x 4 samples. Per core:

  bulk:  fused per (b,h) pipeline overlapping the pred DMA:
         logits = predT.T @ W (bf16) -> E = exp(logits+b) ->
         per-state probability tiles via one-hot gather matmuls into
         TIME-MAJOR tiles PQ/PQB [128, 500, 36] (contiguous DP reads):
           PQ[:, t', :]  ascending rows,  t = t'        (fwd)
           PQB[:, t', :] descending rows, t = 500 + t'  (bwd)
  DP:    linear-domain CTC forward split into TWO independent chains:
           fwd:  X   <- (W1 @ X) .* PQ[k],          k = 1..499
           bwd:  g_t <- W1 @ (g_{t+1} .* PQB[..]),  t = 999..500
         The bwd serial loop (mul -> matmul -> mul) is the critical
         path; the injection matmul (readout one-hot at t == il) is
         HOISTED off it: inj(k) runs start=True early, the W1 matmul
         accumulates start=False.  Halo refresh every 8 steps: PR
         permutation matmul + two VECTOR copies.  Rescale every 128
         steps (fwd k%128==0, bwd k%128==64), logs deferred to one
         final Ln over a [1,4,8] stash.
  meet:  loss_lin = <alpha_499, g_500> via two host-built row-reversal
         matmuls + masked dot; nll = C1*il + ln(e1)*2tl - sum(logs)
         - ln(dot), y = nll/tl.
"""
import numpy as np
import ml_dtypes

B, T, D, V, L = 32, 1000, 768, 31, 200
S = 2 * L + 1
NJ = 9
CORE = 48
HALO = 16
BLK = 64
B_LOC = 4
NCOL = B_LOC * NJ            # 36
C1 = 2.5
E1 = 0.5                     # bf16-exact; E1^2 = 0.25 also bf16-exact.
                             # Tilt tuned so per-step lattice drift ~ 0:
                             # 128-step window sums stay well inside the
                             # Act-engine Ln table domain [2^-64, 2^64].
CTILT = float(np.log(E1))
RESC = 128
REFRESH = 8
NINJ = 200                   # injection window t in [800, 999]
NSLOT = 8                    # 3 fwd rescales + 4 bwd rescales + 1 dot
NRB = 4                      # bwd rescale count
EPS = 1e-30
HT = 500                     # half horizon


def _desc_pos(s):
    """Descending-layout position of state s: (col, row)."""
    jb = (s + HALO) // CORE
    return jb, CORE * jb + 47 - s


def _build_core_consts(tgt2d, in_len, tgt_len, b_base):
    """Host-built constants for one core (samples b_base..b_base+3)."""
    bf = ml_dtypes.bfloat16
    il = in_len[b_base:b_base + B_LOC].astype(np.int64)
    tl = tgt_len[b_base:b_base + B_LOC].astype(np.int64)
    ext = np.zeros((B_LOC, S), np.int64)
    ext[:, 1::2] = tgt2d[b_base:b_base + B_LOC]
    skip = np.zeros((B_LOC, S), bool)
    skip[:, 2:] = (ext[:, 2:] != 0) & (ext[:, 2:] != ext[:, :-2])
    m2s = np.zeros((B_LOC, S + 2), bool)
    m2s[:, :S - 2] = skip[:, 2:]

    # gather one-hots carry the per-frame boost kappa = E1 (bf16-exact);
    # PQ = kappa * E stays unnormalized -- the 1/Z and kappa corrections
    # are reclaimed at readout via sum(lnZ[t<il]) and CTILT*il.
    # ascending (fwd): masked rows use m2s (dest-skip)
    G = np.zeros((V, NCOL, 128), np.float32)
    # descending (bwd): masked rows use skip (source-skip)
    GB = np.zeros((V, NCOL, 128), np.float32)
    for b in range(B_LOC):
        for j in range(NJ):
            for m in range(BLK):
                s = CORE * j - HALO + m
                if 0 <= s <= 2 * int(tl[b]):
                    G[ext[b, s], b * NJ + j, m] = E1
                    if m2s[b, s]:
                        G[ext[b, s], b * NJ + j, 64 + m] = E1
                sd = CORE * j + 47 - m
                if 0 <= sd <= 2 * int(tl[b]):
                    GB[ext[b, sd], b * NJ + j, m] = E1
                    if skip[b, sd]:
                        GB[ext[b, sd], b * NJ + j, 64 + m] = E1

    e1, e2 = E1, E1 * E1
    W1 = np.zeros((128, 128), np.float32)
    for m in range(BLK):
        W1[m, m] = 1.0
        if m >= 1:
            W1[m - 1, m] = e1
        if m >= 2:
            W1[64 + m - 2, m] = e2
        W1[:, 64 + m] = W1[:, m]

    OH01 = np.zeros((128, NCOL), np.float32)
    for b in range(B_LOC):
        c = b * NJ
        OH01[16, c] = 1.0
        OH01[17, c] = e1
        OH01[80, c] = 1.0
        OH01[81, c] = e1

    # halo permutation: rows {48+m, 112+m} -> psum rows {m, 64+m}, m<16
    PR = np.zeros((128, 80), np.float32)
    for m in range(16):
        PR[48 + m, m] = 1.0
        PR[112 + m, 64 + m] = 1.0

    # bwd injection at end states {2tl-1 (w=e1), 2tl (w=1)}, desc coords
    INJ0 = np.zeros((128, NCOL), np.float32)
    VB = np.zeros((2 * B_LOC, 128), np.float32)
    CB = np.zeros((2 * B_LOC, NINJ, NCOL), np.float32)
    for b in range(B_LOC):
        for i, send in enumerate([2 * int(tl[b]) - 1, 2 * int(tl[b])]):
            w = e1 if i == 0 else 1.0
            jb, m = _desc_pos(send)
            if int(il[b]) == 1000:
                INJ0[m, b * NJ + jb] += w
                INJ0[64 + m, b * NJ + jb] += w
            else:
                VB[2 * b + i, m] = w
                VB[2 * b + i, 64 + m] = w
                k = 999 - int(il[b])
                CB[2 * b + i, k, b * NJ + jb] = 1.0

    CM = np.zeros((128, 1), np.float32)
    CM[16:64, 0] = 1.0           # core rows for BOTH layouts

    # meet reversal: fwd row r=16+u (state 48j+u) <- desc row (same col
    # j for u<32: 47-u; col j+1 for u>=32: 95-u)
    REV1 = np.zeros((128, 128), np.float32)
    REV2 = np.zeros((128, 128), np.float32)
    for u in range(32):
        REV1[47 - u, 16 + u] = 1.0
    for u in range(32, 48):
        REV2[95 - u, 16 + u] = 1.0

    # bwd rescale masks per slot: block at round r acts on g_{tb+1},
    # tb = 999 - k_r, k_r = 64 + 128*r: mask = il > tb
    MRB = np.zeros((1, B_LOC, NRB), np.float32)
    MRB1 = np.zeros((1, B_LOC, NRB), np.float32)
    for r in range(NRB):
        tb = 999 - (64 + RESC * r)
        mask = (il > tb).astype(np.float32)
        MRB[0, :, r] = mask
        MRB1[0, :, r] = 1.0 - mask

    # per-sample mask t < il over the [b, h, t'] layout (for sum lnZ_t)
    ILM = np.zeros((1, B_LOC, 2, HT), np.float32)
    for b in range(B_LOC):
        for h in range(2):
            for tp in range(HT):
                if h * HT + tp < int(il[b]):
                    ILM[0, b, h, tp] = 1.0

    C1IL = (CTILT * (il + 2 * tl)).astype(np.float32).reshape(1, B_LOC)
    RTL = (1.0 / tl.astype(np.float64)).astype(np.float32).reshape(1, B_LOC)
    return dict(g=G.reshape(V, NCOL * 128).astype(bf),
                gb=GB.reshape(V, NCOL * 128).astype(bf),
                w1=W1.astype(bf), pr=PR.astype(bf),
                oh01=OH01.astype(bf), inj0=INJ0.astype(bf),
                vb=VB.astype(bf),
                cb=CB.reshape(2 * B_LOC, NINJ * NCOL).astype(bf),
                cm=CM.astype(bf),
                rev1=REV1.astype(bf), rev2=REV2.astype(bf),
                mrb=MRB.reshape(1, B_LOC * NRB),
                mrb1=MRB1.reshape(1, B_LOC * NRB),
                ilm=ILM.reshape(1, B_LOC * 2 * HT).astype(bf),
                c1il=C1IL, rtl=RTL)


def build_program():
    import concourse.bacc as bacc
    import concourse.tile as tile
    from concourse import mybir

    f32 = mybir.dt.float32
    bf16 = mybir.dt.bfloat16
    AF = mybir.ActivationFunctionType
    ALU = mybir.AluOpType
    AX = mybir.AxisListType

    nc = bacc.Bacc("TRN2", target_bir_lowering=False, debug=False,
                   enable_asserts=False)

    predt = nc.dram_tensor("predt", [B_LOC, D, T], bf16, kind="ExternalInput").ap()
    wt = nc.dram_tensor("wt", [D, V], bf16, kind="ExternalInput").ap()
    bb = nc.dram_tensor("bb", [V, 1], f32, kind="ExternalInput").ap()
    g_d = nc.dram_tensor("g", [V, NCOL * 128], bf16, kind="ExternalInput").ap()
    gb_d = nc.dram_tensor("gb", [V, NCOL * 128], bf16, kind="ExternalInput").ap()
    w1_d = nc.dram_tensor("w1", [128, 128], bf16, kind="ExternalInput").ap()
    pr_d = nc.dram_tensor("pr", [128, 80], bf16, kind="ExternalInput").ap()
    oh01_d = nc.dram_tensor("oh01", [128, NCOL], bf16, kind="ExternalInput").ap()
    inj0_d = nc.dram_tensor("inj0", [128, NCOL], bf16, kind="ExternalInput").ap()
    vb_d = nc.dram_tensor("vb", [2 * B_LOC, 128], bf16, kind="ExternalInput").ap()
    cb_d = nc.dram_tensor("cb", [2 * B_LOC, NINJ * NCOL], bf16, kind="ExternalInput").ap()
    cm_d = nc.dram_tensor("cm", [128, 1], bf16, kind="ExternalInput").ap()
    rev1_d = nc.dram_tensor("rev1", [128, 128], bf16, kind="ExternalInput").ap()
    rev2_d = nc.dram_tensor("rev2", [128, 128], bf16, kind="ExternalInput").ap()
    mrb_d = nc.dram_tensor("mrb", [1, B_LOC * NRB], f32, kind="ExternalInput").ap()
    mrb1_d = nc.dram_tensor("mrb1", [1, B_LOC * NRB], f32, kind="ExternalInput").ap()
    ilm_d = nc.dram_tensor("ilm", [1, B_LOC * 2 * HT], bf16, kind="ExternalInput").ap()
    c1il_d = nc.dram_tensor("c1il", [1, B_LOC], f32, kind="ExternalInput").ap()
    rtl_d = nc.dram_tensor("rtl", [1, B_LOC], f32, kind="ExternalInput").ap()
    y_d = nc.dram_tensor("y", [B_LOC, 1], f32, kind="ExternalOutput").ap()

    dma = nc.default_dma_engine.dma_start

    with tile.TileContext(nc) as tc:
        with (
            tc.tile_pool(name="consts", bufs=1) as cp,
            tc.tile_pool(name="persist", bufs=1) as pp,
            tc.tile_pool(name="xs", bufs=3) as xp,
            tc.tile_pool(name="ys", bufs=3) as yp,
            tc.tile_pool(name="small", bufs=4) as smp,
        ):
            # wt first on its own (otherwise idle) queue so the first
            # logits matmul is gated only by pred[b=0]'s arrival.
            # Only sync + gpsimd issue DMAs: the tile scheduler charges a
            # dma_start's full transfer time to the issuing engine, so any
            # queue with bulk compute (Act does EXP + gather copies) would
            # stall that compute behind the transfers.
            wt_sb = cp.tile([128, 6, V], bf16, tag="wt")
            bb_sb = cp.tile([V, 1], f32, tag="bb")
            g_sbf = cp.tile([V, NCOL * 128], bf16, tag="g")
            gb_sbf = cp.tile([V, NCOL * 128], bf16, tag="gb")
            nc.sync.dma_start(wt_sb, wt.rearrange("(c p) v -> p c v", p=128))
            nc.gpsimd.dma_start(bb_sb, bb)
            nc.gpsimd.dma_start(gb_sbf, gb_d)

            predp_cm = tc.tile_pool(name="predp", bufs=4)
            predp = predp_cm.__enter__()
            pts = []
            for b in range(B_LOC):
                pt = predp.tile([128, 6, T], bf16, tag="pt")
                pts.append(pt)
            # b0 per-chunk on sync: the (b0,h,c) logits matmul starts as
            # soon as chunk c lands instead of waiting for the full 1.5 MB
            pr0 = predt[0].rearrange("(c p) t -> c p t", p=128)
            for c in range(6):
                nc.sync.dma_start(pts[0][:, c, :], pr0[c])
            nc.sync.dma_start(g_sbf, g_d)
            nc.sync.dma_start(pts[1],
                              predt[1].rearrange("(c p) t -> p c t", p=128))
            nc.gpsimd.dma_start(pts[2],
                                predt[2].rearrange("(c p) t -> p c t", p=128))
            nc.gpsimd.dma_start(pts[3],
                                predt[3].rearrange("(c p) t -> p c t", p=128))
            w1_sb = cp.tile([128, 128], bf16, tag="w1")
            pr_sb = cp.tile([128, 80], bf16, tag="pr")
            oh01_sb = cp.tile([128, NCOL], bf16, tag="oh01")
            inj0_sb = cp.tile([128, NCOL], bf16, tag="inj0")
            vb_sb = cp.tile([2 * B_LOC, 128], bf16, tag="vb")
            cb_sbf = cp.tile([2 * B_LOC, NINJ * NCOL], bf16, tag="cb")
            cm_sb = cp.tile([128, 1], bf16, tag="cm")
            rev1_sb = cp.tile([128, 128], bf16, tag="rev1")
            rev2_sb = cp.tile([128, 128], bf16, tag="rev2")
            mrb_sb = cp.tile([1, B_LOC, NRB], f32, tag="mrb")
            mrb1_sb = cp.tile([1, B_LOC, NRB], f32, tag="mrb1")
            ilm_sb = cp.tile([1, B_LOC, 2, HT], bf16, tag="ilm")
            c1il_sb = cp.tile([1, B_LOC], f32, tag="c1il")
            rtl_sb = cp.tile([1, B_LOC], f32, tag="rtl")

            nc.gpsimd.dma_start(w1_sb, w1_d)
            nc.gpsimd.dma_start(pr_sb, pr_d)
            nc.gpsimd.dma_start(oh01_sb, oh01_d)
            nc.gpsimd.dma_start(inj0_sb, inj0_d)
            nc.gpsimd.dma_start(vb_sb, vb_d)
            nc.gpsimd.dma_start(cb_sbf, cb_d)
            nc.gpsimd.dma_start(cm_sb, cm_d)
            nc.gpsimd.dma_start(rev1_sb, rev1_d)
            nc.gpsimd.dma_start(rev2_sb, rev2_d)
            nc.gpsimd.dma_start(mrb_sb, mrb_d.rearrange("p (b k) -> p b k",
                                                        k=NRB))
            nc.gpsimd.dma_start(mrb1_sb, mrb1_d.rearrange("p (b k) -> p b k",
                                                          k=NRB))
            nc.gpsimd.dma_start(ilm_sb, ilm_d.rearrange("p (b h t) -> p b h t",
                                                        h=2, t=HT))
            nc.gpsimd.dma_start(c1il_sb, c1il_d)
            nc.gpsimd.dma_start(rtl_sb, rtl_d)
            g_sb = g_sbf.rearrange("v (c m) -> v c m", m=128)
            gb_sb = gb_sbf.rearrange("v (c m) -> v c m", m=128)
            cb_sb = cb_sbf.rearrange("p (k c) -> p k c", c=NCOL)

            ones_colV = cp.tile([V, 1], bf16, tag="ones_colV")
            nc.vector.memset(ones_colV, 1.0)
            ones_row128f = cp.tile([1, 128], f32, tag="ones_row128f")
            nc.vector.memset(ones_row128f, 1.0)
            ones128 = cp.tile([128, 1], bf16, tag="ones128")
            nc.vector.memset(ones128, 1.0)

            E_sb = pp.tile([V, B_LOC, 2, HT], bf16, tag="E")
            ZS = pp.tile([1, B_LOC, 2, HT], bf16, tag="ZS")
            ZL = pp.tile([1, B_LOC, 2, HT], bf16, tag="ZL")
            ZM = pp.tile([1, B_LOC, 2, HT], bf16, tag="ZM")
            SZ = pp.tile([1, B_LOC], f32, tag="SZ")
            PQ = pp.tile([128, NCOL, HT], bf16, tag="PQ")
            PQB = pp.tile([128, NCOL, HT], bf16, tag="PQB")
            SLOG = pp.tile([1, B_LOC, NSLOT], f32, tag="SLOG")
            nc.vector.memset(SLOG, 1.0)

            # ---------------- bulk: fused logits -> E -> gathers ------------
            with (
                tc.tile_pool(name="plog", bufs=2, space="PSUM") as plog,
                tc.tile_pool(name="pz", bufs=2, space="PSUM") as pzp,
                tc.tile_pool(name="pgat", bufs=3, space="PSUM") as pgatp,
            ):
                cp_eng = [nc.scalar, nc.vector, nc.scalar]
                for b in range(B_LOC):
                    pt = pts[b]
                    for h in range(2):
                        ps_log = plog.tile([V, HT], f32, tag="pslog")
                        for c in range(6):
                            nc.tensor.matmul(ps_log, wt_sb[:, c, :],
                                             pt[:, c, h * HT:(h + 1) * HT],
                                             start=(c == 0), stop=(c == 5))
                        Es = E_sb[:, b, h, :]
                        nc.scalar.activation(Es, ps_log, AF.Exp, bias=bb_sb)
                        ps_z = pzp.tile([1, HT], f32, tag="psz")
                        nc.tensor.matmul(ps_z, ones_colV, Es,
                                         start=True, stop=True)
                        nc.vector.tensor_copy(ZS[:, b, h, :], ps_z)
                        gsel = g_sb if h == 0 else gb_sb
                        dstq = PQ if h == 0 else PQB
                        for j in range(NJ):
                            gidx = b * NJ + j
                            ps_g = pgatp.tile([128, HT], f32, tag="psg")
                            nc.tensor.matmul(ps_g, gsel[:, gidx, :],
                                             E_sb[:, b, h, :],
                                             start=True, stop=True)
                            dst = dstq[:, gidx, :]
                            if j % 3 == 2:
                                nc.vector.tensor_copy(dst, ps_g)
                            else:
                                nc.scalar.copy(dst, ps_g)
                predp_cm.__exit__(None, None, None)

                # lnZ correction (only gates final readout; overlaps DP)
                nc.scalar.activation(ZL.rearrange("p b h t -> p (b h t)"),
                                     ZS.rearrange("p b h t -> p (b h t)"),
                                     AF.Ln)
                nc.gpsimd.tensor_mul(ZM, ZL, ilm_sb)
                for b in range(B_LOC):
                    nc.scalar.activation(
                        ZL[:, b, :, :].rearrange("p h t -> p (h t)"),
                        ZM[:, b, :, :].rearrange("p h t -> p (h t)"),
                        AF.Copy, accum_out=SZ[:, b:b + 1])

            # ---------------- DP: interleaved fwd + bwd chains --------------
            with (
                tc.tile_pool(name="ppf", bufs=2, space="PSUM") as ppf,
                tc.tile_pool(name="ppb", bufs=2, space="PSUM") as ppb,
                tc.tile_pool(name="prs", bufs=1, space="PSUM") as prs,
                tc.tile_pool(name="pbr", bufs=1, space="PSUM") as pbr,
                tc.tile_pool(name="phalo", bufs=2, space="PSUM") as php,
            ):
                fslot = [0]
                bslot = [3]

                def rescale(state, pool, is_bwd):
                    ps36 = prs.tile([1, NCOL], f32, tag="ps36")
                    nc.tensor.matmul(ps36, cm_sb, state, start=True, stop=True)
                    s4 = smp.tile([1, B_LOC], f32, tag="s4")
                    nc.vector.tensor_reduce(
                        s4, ps36.rearrange("p (b j) -> p b j", j=NJ),
                        axis=AX.X, op=ALU.add)
                    if is_bwd:
                        k = bslot[0]; bslot[0] += 1
                        sm = smp.tile([1, B_LOC], f32, tag="sm")
                        nc.vector.tensor_mul(sm, s4, mrb_sb[:, :, k - 3])
                        nc.vector.tensor_add(SLOG[:, :, k], sm,
                                             mrb1_sb[:, :, k - 3])
                        s4e = smp.tile([1, B_LOC], f32, tag="s4e")
                        nc.vector.tensor_scalar_add(s4e, s4, EPS)
                        s4 = s4e
                    else:
                        k = fslot[0]; fslot[0] += 1
                        nc.scalar.copy(SLOG[:, :, k], s4)
                    rz4 = smp.tile([1, B_LOC], f32, tag="rz4")
                    nc.vector.reciprocal(rz4, s4)
                    psbr = pbr.tile([128, B_LOC], f32, tag="psbr")
                    nc.tensor.matmul(psbr, ones_row128f, rz4,
                                     start=True, stop=True)
                    out = pool.tile([128, NCOL], bf16,
                                    tag="Y" if is_bwd else "X")
                    for b in range(B_LOC):
                        nc.vector.tensor_scalar_mul(
                            out[:, b * NJ:(b + 1) * NJ],
                            state[:, b * NJ:(b + 1) * NJ], psbr[:, b:b + 1])
                    return out

                # init fwd X; init bwd round 0: Y = inj0 * PQB[499] (t=999)
                X = xp.tile([128, NCOL], bf16, tag="X")
                nc.vector.tensor_mul(X, PQ[:, :, 0], oh01_sb)
                Y = yp.tile([128, NCOL], bf16, tag="Y")
                nc.vector.tensor_mul(Y, inj0_sb, PQB[:, :, HT - 1])
                psb = ppb.tile([128, NCOL], f32, tag="psb")
                nc.tensor.matmul(psb, vb_sb, cb_sb[:, 0, :],
                                 start=True, stop=False)
                nc.tensor.matmul(psb, w1_sb, Y, start=False, stop=True)

                for k in range(1, HT):
                    # ---- bwd step t = 999 - k (k = 1..499 -> t=998..500) ----
                    # injection matmul first (start=True) so it runs OFF the
                    # bwd serial loop; the W1 matmul accumulates into it.
                    inj = k < NINJ
                    psb_n = ppb.tile([128, NCOL], f32, tag="psb")
                    if inj:
                        nc.tensor.matmul(psb_n, vb_sb, cb_sb[:, k, :],
                                         start=True, stop=False)
                    Y = yp.tile([128, NCOL], bf16, tag="Y")
                    nc.vector.tensor_mul(Y, psb, PQB[:, :, HT - 1 - k])
                    if k % REFRESH == 4:
                        yv = Y.rearrange("p (b j) -> p b j", j=NJ)
                        psh2 = php.tile([80, 32], f32, tag="psh")
                        nc.tensor.matmul(psh2, pr_sb, yv[:, :, 1:9],
                                         start=True, stop=True)
                        ph2v = psh2.rearrange("p (b j) -> p b j", j=8)
                        nc.vector.tensor_copy(yv[0:16, :, 0:8], ph2v[0:16])
                        nc.vector.tensor_copy(yv[64:80, :, 0:8], ph2v[64:80])
                    if k % RESC == 64:
                        Y = rescale(Y, yp, True)
                    nc.tensor.matmul(psb_n, w1_sb, Y,
                                     start=not inj, stop=True)
                    psb = psb_n

                    # ---- fwd step t = k (k = 1..499) ----
                    if k % RESC == 0:
                        X = rescale(X, xp, False)
                    psf = ppf.tile([128, NCOL], f32, tag="psf")
                    nc.tensor.matmul(psf, w1_sb, X, start=True, stop=True)
                    Xn = xp.tile([128, NCOL], bf16, tag="X")
                    nc.vector.tensor_mul(Xn, psf, PQ[:, :, k])
                    if k % REFRESH == 0:
                        xv = Xn.rearrange("p (b j) -> p b j", j=NJ)
                        psh = php.tile([80, 32], f32, tag="psh")
                        nc.tensor.matmul(psh, pr_sb, xv[:, :, 0:8],
                                         start=True, stop=True)
                        phv = psh.rearrange("p (b j) -> p b j", j=8)
                        nc.vector.tensor_copy(xv[0:16, :, 1:9], phv[0:16])
                        nc.vector.tensor_copy(xv[64:80, :, 1:9], phv[64:80])
                    X = Xn

                # ---------------- meet + readout ----------------------------
                Gs = smp.tile([128, NCOL], bf16, tag="Gs")
                nc.vector.tensor_copy(Gs, psb)
                gf1 = ppf.tile([128, NCOL], f32, tag="psf")
                nc.tensor.matmul(gf1, rev1_sb, Gs, start=True, stop=True)
                gf2 = ppb.tile([128, NCOL], f32, tag="psb")
                nc.tensor.matmul(gf2, rev2_sb, Gs, start=True, stop=True)
                D1 = smp.tile([128, NCOL], bf16, tag="D1")
                nc.vector.tensor_mul(D1, X, gf1)
                D2 = smp.tile([128, B_LOC, 8], bf16, tag="D2")
                xvw = X.rearrange("p (b j) -> p b j", j=NJ)
                gf2v = gf2.rearrange("p (b j) -> p b j", j=NJ)
                nc.vector.tensor_mul(D2, xvw[:, :, 0:8], gf2v[:, :, 1:9])
                ps1 = prs.tile([1, NCOL], f32, tag="ps36")
                nc.tensor.matmul(ps1, ones128, D1, start=True, stop=False,
                                 skip_group_check=True)
                p1v = ps1.rearrange("p (b j) -> p b j", j=NJ)
                nc.tensor.matmul(p1v[:, :, 0:8], ones128, D2,
                                 start=False, stop=True,
                                 skip_group_check=True)
                nc.vector.tensor_reduce(
                    SLOG[:, :, NSLOT - 1], p1v, axis=AX.X, op=ALU.add)
                logs = smp.tile([1, B_LOC, NSLOT], f32, tag="logs")
                nc.scalar.activation(logs.rearrange("p b k -> p (b k)"),
                                     SLOG.rearrange("p b k -> p (b k)"),
                                     AF.Ln)
                tot = smp.tile([1, B_LOC], f32, tag="tot")
                nc.vector.tensor_reduce(tot, logs, axis=AX.X, op=ALU.add)
                t1 = smp.tile([1, B_LOC], f32, tag="t1")
                nc.vector.tensor_sub(t1, c1il_sb, tot)
                nll = smp.tile([1, B_LOC], f32, tag="nll")
                nc.vector.tensor_add(nll, t1, SZ)
                yv_ = smp.tile([1, B_LOC], f32, tag="yv")
                nc.vector.tensor_mul(yv_, nll, rtl_sb)
                dma(y_d.rearrange("b one -> one b"), yv_)

    nc.compile()
    return nc


def build_in_maps(inputs):
    """Shard inputs + host-built constants -> one in_map per core."""
    bf = ml_dtypes.bfloat16
    pred = np.asarray(inputs["pred"], np.float32)
    targets = np.asarray(inputs["targets"]).astype(np.int64)
    in_len = np.asarray(inputs["input_lengths"]).astype(np.int64)
    tgt_len = np.asarray(inputs["target_lengths"]).astype(np.int64)
    Wm = np.asarray(inputs["W"], np.float32)
    bv = np.asarray(inputs["b"], np.float32)
    tgt2d = targets.reshape(B, L)
    predt_all = np.ascontiguousarray(pred.transpose(0, 2, 1)).astype(bf)
    wt = np.ascontiguousarray(Wm.T).astype(bf)
    bb = np.ascontiguousarray(bv.reshape(V, 1))
    in_maps = []
    for core in range(8):
        b0 = core * B_LOC
        cst = _build_core_consts(tgt2d, in_len, tgt_len, b0)
        im = dict(predt=np.ascontiguousarray(predt_all[b0:b0 + B_LOC]),
                  wt=wt, bb=bb)
        for k, v in cst.items():
            im[k] = np.ascontiguousarray(v)
        in_maps.append(im)
    return in_maps


_CACHED = {}


def kernel(**inputs):
    from concourse import bass_utils
    if "nc" not in _CACHED:
        _CACHED["nc"] = build_program()
    nc = _CACHED["nc"]
    in_maps = build_in_maps(inputs)
    res = bass_utils.run_bass_kernel_spmd(nc, in_maps, core_ids=list(range(8)))
    ys = [r["y"] for r in res.results]
    loss = np.concatenate([y.ravel() for y in ys]).astype(np.float64).sum() / B
    return np.float32(loss)


# revision 18
# speedup vs baseline: 1.1498x; 1.1498x over previous
"""Trainium2 Bass kernel for nn_CTCConsistencyLoss_7310034338203.

Data-parallel over batch: 8 cores x 4 samples. Per core:

  bulk:  fused per (b,h) pipeline overlapping the pred DMA:
         logits = predT.T @ W (bf16) -> E = exp(logits+b) ->
         per-state probability tiles via one-hot gather matmuls into
         TIME-MAJOR tiles PQ/PQB [128, 500, 36] (contiguous DP reads):
           PQ[:, t', :]  ascending rows,  t = t'        (fwd)
           PQB[:, t', :] descending rows, t = 500 + t'  (bwd)
  DP:    linear-domain CTC forward split into TWO independent chains:
           fwd:  X   <- (W1 @ X) .* PQ[k],          k = 1..499
           bwd:  g_t <- W1 @ (g_{t+1} .* PQB[..]),  t = 999..500
         The bwd serial loop (mul -> matmul -> mul) is the critical
         path; the injection matmul (readout one-hot at t == il) is
         HOISTED off it: inj(k) runs start=True early, the W1 matmul
         accumulates start=False.  Halo refresh every 8 steps: PR
         permutation matmul + two VECTOR copies.  Rescale every 128
         steps (fwd k%128==0, bwd k%128==64), logs deferred to one
         final Ln over a [1,4,8] stash.
  meet:  loss_lin = <alpha_499, g_500> via two host-built row-reversal
         matmuls + masked dot; nll = C1*il + ln(e1)*2tl - sum(logs)
         - ln(dot), y = nll/tl.
"""
import numpy as np
import ml_dtypes

B, T, D, V, L = 32, 1000, 768, 31, 200
S = 2 * L + 1
NJ = 9
CORE = 48
HALO = 16
BLK = 64
B_LOC = 4
NCOL = B_LOC * NJ            # 36
C1 = 2.5
E1 = 0.5                     # bf16-exact; E1^2 = 0.25 also bf16-exact.
                             # Tilt tuned so per-step lattice drift ~ 0:
                             # 128-step window sums stay well inside the
                             # Act-engine Ln table domain [2^-64, 2^64].
CTILT = float(np.log(E1))
RESC = 128
REFRESH = 8
NINJ = 200                   # injection window t in [800, 999]
NSLOT = 8                    # 3 fwd rescales + 4 bwd rescales + 1 dot
NRB = 4                      # bwd rescale count
EPS = 1e-30
HT = 500                     # half horizon


def _desc_pos(s):
    """Descending-layout position of state s: (col, row)."""
    jb = (s + HALO) // CORE
    return jb, CORE * jb + 47 - s


def _build_core_consts(tgt2d, in_len, tgt_len, b_base):
    """Host-built constants for one core (samples b_base..b_base+3)."""
    bf = ml_dtypes.bfloat16
    il = in_len[b_base:b_base + B_LOC].astype(np.int64)
    tl = tgt_len[b_base:b_base + B_LOC].astype(np.int64)
    ext = np.zeros((B_LOC, S), np.int64)
    ext[:, 1::2] = tgt2d[b_base:b_base + B_LOC]
    skip = np.zeros((B_LOC, S), bool)
    skip[:, 2:] = (ext[:, 2:] != 0) & (ext[:, 2:] != ext[:, :-2])
    m2s = np.zeros((B_LOC, S + 2), bool)
    m2s[:, :S - 2] = skip[:, 2:]

    # gather one-hots carry the per-frame boost kappa = E1 (bf16-exact);
    # PQ = kappa * E stays unnormalized -- the 1/Z and kappa corrections
    # are reclaimed at readout via sum(lnZ[t<il]) and CTILT*il.
    # ascending (fwd): masked rows use m2s (dest-skip)
    G = np.zeros((V, NCOL, 128), np.float32)
    # descending (bwd): masked rows use skip (source-skip)
    GB = np.zeros((V, NCOL, 128), np.float32)
    for b in range(B_LOC):
        for j in range(NJ):
            for m in range(BLK):
                s = CORE * j - HALO + m
                if 0 <= s <= 2 * int(tl[b]):
                    G[ext[b, s], b * NJ + j, m] = E1
                    if m2s[b, s]:
                        G[ext[b, s], b * NJ + j, 64 + m] = E1
                sd = CORE * j + 47 - m
                if 0 <= sd <= 2 * int(tl[b]):
                    GB[ext[b, sd], b * NJ + j, m] = E1
                    if skip[b, sd]:
                        GB[ext[b, sd], b * NJ + j, 64 + m] = E1

    e1, e2 = E1, E1 * E1
    W1 = np.zeros((128, 128), np.float32)
    for m in range(BLK):
        W1[m, m] = 1.0
        if m >= 1:
            W1[m - 1, m] = e1
        if m >= 2:
            W1[64 + m - 2, m] = e2
        W1[:, 64 + m] = W1[:, m]

    OH01 = np.zeros((128, NCOL), np.float32)
    for b in range(B_LOC):
        c = b * NJ
        OH01[16, c] = 1.0
        OH01[17, c] = e1
        OH01[80, c] = 1.0
        OH01[81, c] = e1

    # halo permutation: rows {48+m, 112+m} -> psum rows {m, 64+m}, m<16
    PR = np.zeros((128, 80), np.float32)
    for m in range(16):
        PR[48 + m, m] = 1.0
        PR[112 + m, 64 + m] = 1.0

    # bwd injection at end states {2tl-1 (w=e1), 2tl (w=1)}, desc coords
    INJ0 = np.zeros((128, NCOL), np.float32)
    VB = np.zeros((2 * B_LOC, 128), np.float32)
    CB = np.zeros((2 * B_LOC, NINJ, NCOL), np.float32)
    for b in range(B_LOC):
        for i, send in enumerate([2 * int(tl[b]) - 1, 2 * int(tl[b])]):
            w = e1 if i == 0 else 1.0
            jb, m = _desc_pos(send)
            if int(il[b]) == 1000:
                INJ0[m, b * NJ + jb] += w
                INJ0[64 + m, b * NJ + jb] += w
            else:
                VB[2 * b + i, m] = w
                VB[2 * b + i, 64 + m] = w
                k = 999 - int(il[b])
                CB[2 * b + i, k, b * NJ + jb] = 1.0

    CM = np.zeros((128, 1), np.float32)
    CM[16:64, 0] = 1.0           # core rows for BOTH layouts

    # meet reversal: fwd row r=16+u (state 48j+u) <- desc row (same col
    # j for u<32: 47-u; col j+1 for u>=32: 95-u)
    REV1 = np.zeros((128, 128), np.float32)
    REV2 = np.zeros((128, 128), np.float32)
    for u in range(32):
        REV1[47 - u, 16 + u] = 1.0
    for u in range(32, 48):
        REV2[95 - u, 16 + u] = 1.0

    # bwd rescale masks per slot: block at round r acts on g_{tb+1},
    # tb = 999 - k_r, k_r = 64 + 128*r: mask = il > tb
    MRB = np.zeros((1, B_LOC, NRB), np.float32)
    MRB1 = np.zeros((1, B_LOC, NRB), np.float32)
    for r in range(NRB):
        tb = 999 - (64 + RESC * r)
        mask = (il > tb).astype(np.float32)
        MRB[0, :, r] = mask
        MRB1[0, :, r] = 1.0 - mask

    # per-sample mask t < il over the [b, h, t'] layout (for sum lnZ_t)
    ILM = np.zeros((1, B_LOC, 2, HT), np.float32)
    for b in range(B_LOC):
        for h in range(2):
            for tp in range(HT):
                if h * HT + tp < int(il[b]):
                    ILM[0, b, h, tp] = 1.0

    C1IL = (CTILT * (il + 2 * tl)).astype(np.float32).reshape(1, B_LOC)
    RTL = (1.0 / tl.astype(np.float64)).astype(np.float32).reshape(1, B_LOC)
    return dict(g=G.reshape(V, NCOL * 128).astype(bf),
                gb=GB.reshape(V, NCOL * 128).astype(bf),
                w1=W1.astype(bf), pr=PR.astype(bf),
                oh01=OH01.astype(bf), inj0=INJ0.astype(bf),
                vb=VB.astype(bf),
                cb=CB.reshape(2 * B_LOC, NINJ * NCOL).astype(bf),
                cm=CM.astype(bf),
                rev1=REV1.astype(bf), rev2=REV2.astype(bf),
                mrb=MRB.reshape(1, B_LOC * NRB),
                mrb1=MRB1.reshape(1, B_LOC * NRB),
                ilm=ILM.reshape(1, B_LOC * 2 * HT).astype(bf),
                c1il=C1IL, rtl=RTL)


def build_program():
    import concourse.bacc as bacc
    import concourse.tile as tile
    from concourse import mybir

    f32 = mybir.dt.float32
    bf16 = mybir.dt.bfloat16
    AF = mybir.ActivationFunctionType
    ALU = mybir.AluOpType
    AX = mybir.AxisListType

    nc = bacc.Bacc("TRN2", target_bir_lowering=False, debug=False,
                   enable_asserts=False)

    predt = nc.dram_tensor("predt", [B_LOC, D, T], bf16, kind="ExternalInput").ap()
    wt = nc.dram_tensor("wt", [D, V], bf16, kind="ExternalInput").ap()
    bb = nc.dram_tensor("bb", [V, 1], f32, kind="ExternalInput").ap()
    g_d = nc.dram_tensor("g", [V, NCOL * 128], bf16, kind="ExternalInput").ap()
    gb_d = nc.dram_tensor("gb", [V, NCOL * 128], bf16, kind="ExternalInput").ap()
    w1_d = nc.dram_tensor("w1", [128, 128], bf16, kind="ExternalInput").ap()
    pr_d = nc.dram_tensor("pr", [128, 80], bf16, kind="ExternalInput").ap()
    oh01_d = nc.dram_tensor("oh01", [128, NCOL], bf16, kind="ExternalInput").ap()
    inj0_d = nc.dram_tensor("inj0", [128, NCOL], bf16, kind="ExternalInput").ap()
    vb_d = nc.dram_tensor("vb", [2 * B_LOC, 128], bf16, kind="ExternalInput").ap()
    cb_d = nc.dram_tensor("cb", [2 * B_LOC, NINJ * NCOL], bf16, kind="ExternalInput").ap()
    cm_d = nc.dram_tensor("cm", [128, 1], bf16, kind="ExternalInput").ap()
    rev1_d = nc.dram_tensor("rev1", [128, 128], bf16, kind="ExternalInput").ap()
    rev2_d = nc.dram_tensor("rev2", [128, 128], bf16, kind="ExternalInput").ap()
    mrb_d = nc.dram_tensor("mrb", [1, B_LOC * NRB], f32, kind="ExternalInput").ap()
    mrb1_d = nc.dram_tensor("mrb1", [1, B_LOC * NRB], f32, kind="ExternalInput").ap()
    ilm_d = nc.dram_tensor("ilm", [1, B_LOC * 2 * HT], bf16, kind="ExternalInput").ap()
    c1il_d = nc.dram_tensor("c1il", [1, B_LOC], f32, kind="ExternalInput").ap()
    rtl_d = nc.dram_tensor("rtl", [1, B_LOC], f32, kind="ExternalInput").ap()
    y_d = nc.dram_tensor("y", [B_LOC, 1], f32, kind="ExternalOutput").ap()

    dma = nc.default_dma_engine.dma_start

    with tile.TileContext(nc) as tc:
        with (
            tc.tile_pool(name="consts", bufs=1) as cp,
            tc.tile_pool(name="persist", bufs=1) as pp,
            tc.tile_pool(name="xs", bufs=3) as xp,
            tc.tile_pool(name="ys", bufs=3) as yp,
            tc.tile_pool(name="small", bufs=4) as smp,
        ):
            # wt first on its own (otherwise idle) queue so the first
            # logits matmul is gated only by pred[b=0]'s arrival.
            # Only sync + gpsimd issue DMAs: the tile scheduler charges a
            # dma_start's full transfer time to the issuing engine, so any
            # queue with bulk compute (Act does EXP + gather copies) would
            # stall that compute behind the transfers.
            wt_sb = cp.tile([128, 6, V], bf16, tag="wt")
            bb_sb = cp.tile([V, 1], f32, tag="bb")
            g_sbf = cp.tile([V, NCOL * 128], bf16, tag="g")
            gb_sbf = cp.tile([V, NCOL * 128], bf16, tag="gb")

            predp_cm = tc.tile_pool(name="predp", bufs=4)
            predp = predp_cm.__enter__()
            pts = []
            for b in range(B_LOC):
                pt = predp.tile([128, 6, T], bf16, tag="pt")
                pts.append(pt)
            # sync ring: b0 first (gates the first logits), then g, b2.
            # gpsimd ring: small consts first, then b1, b3.
            nc.sync.dma_start(pts[0],
                              predt[0].rearrange("(c p) t -> p c t", p=128))
            nc.gpsimd.dma_start(wt_sb, wt.rearrange("(c p) v -> p c v", p=128))
            nc.gpsimd.dma_start(bb_sb, bb)
            nc.gpsimd.dma_start(gb_sbf, gb_d)
            nc.sync.dma_start(g_sbf, g_d)
            nc.gpsimd.dma_start(pts[1],
                                predt[1].rearrange("(c p) t -> p c t", p=128))
            nc.sync.dma_start(pts[2],
                              predt[2].rearrange("(c p) t -> p c t", p=128))
            nc.gpsimd.dma_start(pts[3],
                                predt[3].rearrange("(c p) t -> p c t", p=128))
            w1_sb = cp.tile([128, 128], bf16, tag="w1")
            pr_sb = cp.tile([128, 80], bf16, tag="pr")
            oh01_sb = cp.tile([128, NCOL], bf16, tag="oh01")
            inj0_sb = cp.tile([128, NCOL], bf16, tag="inj0")
            vb_sb = cp.tile([2 * B_LOC, 128], bf16, tag="vb")
            cb_sbf = cp.tile([2 * B_LOC, NINJ * NCOL], bf16, tag="cb")
            cm_sb = cp.tile([128, 1], bf16, tag="cm")
            rev1_sb = cp.tile([128, 128], bf16, tag="rev1")
            rev2_sb = cp.tile([128, 128], bf16, tag="rev2")
            mrb_sb = cp.tile([1, B_LOC, NRB], f32, tag="mrb")
            mrb1_sb = cp.tile([1, B_LOC, NRB], f32, tag="mrb1")
            ilm_sb = cp.tile([1, B_LOC, 2, HT], bf16, tag="ilm")
            c1il_sb = cp.tile([1, B_LOC], f32, tag="c1il")
            rtl_sb = cp.tile([1, B_LOC], f32, tag="rtl")

            nc.gpsimd.dma_start(w1_sb, w1_d)
            nc.gpsimd.dma_start(pr_sb, pr_d)
            nc.gpsimd.dma_start(oh01_sb, oh01_d)
            nc.gpsimd.dma_start(inj0_sb, inj0_d)
            nc.gpsimd.dma_start(vb_sb, vb_d)
            nc.gpsimd.dma_start(cb_sbf, cb_d)
            nc.gpsimd.dma_start(cm_sb, cm_d)
            nc.gpsimd.dma_start(rev1_sb, rev1_d)
            nc.gpsimd.dma_start(rev2_sb, rev2_d)
            nc.gpsimd.dma_start(mrb_sb, mrb_d.rearrange("p (b k) -> p b k",
                                                        k=NRB))
            nc.gpsimd.dma_start(mrb1_sb, mrb1_d.rearrange("p (b k) -> p b k",
                                                          k=NRB))
            nc.gpsimd.dma_start(ilm_sb, ilm_d.rearrange("p (b h t) -> p b h t",
                                                        h=2, t=HT))
            nc.gpsimd.dma_start(c1il_sb, c1il_d)
            nc.gpsimd.dma_start(rtl_sb, rtl_d)
            g_sb = g_sbf.rearrange("v (c m) -> v c m", m=128)
            gb_sb = gb_sbf.rearrange("v (c m) -> v c m", m=128)
            cb_sb = cb_sbf.rearrange("p (k c) -> p k c", c=NCOL)

            ones_colV = cp.tile([V, 1], bf16, tag="ones_colV")
            nc.vector.memset(ones_colV, 1.0)
            ones_row128f = cp.tile([1, 128], f32, tag="ones_row128f")
            nc.vector.memset(ones_row128f, 1.0)
            ones128 = cp.tile([128, 1], bf16, tag="ones128")
            nc.vector.memset(ones128, 1.0)

            E_sb = pp.tile([V, B_LOC, 2, HT], bf16, tag="E")
            ZS = pp.tile([1, B_LOC, 2, HT], bf16, tag="ZS")
            ZL = pp.tile([1, B_LOC, 2, HT], bf16, tag="ZL")
            ZM = pp.tile([1, B_LOC, 2, HT], bf16, tag="ZM")
            SZ = pp.tile([1, B_LOC], f32, tag="SZ")
            PQ = pp.tile([128, NCOL, HT], bf16, tag="PQ")
            PQB = pp.tile([128, NCOL, HT], bf16, tag="PQB")
            SLOG = pp.tile([1, B_LOC, NSLOT], f32, tag="SLOG")
            nc.vector.memset(SLOG, 1.0)

            # ---------------- bulk: fused logits -> E -> gathers ------------
            with (
                tc.tile_pool(name="plog", bufs=2, space="PSUM") as plog,
                tc.tile_pool(name="pz", bufs=2, space="PSUM") as pzp,
                tc.tile_pool(name="pgat", bufs=3, space="PSUM") as pgatp,
            ):
                cp_eng = [nc.scalar, nc.vector, nc.scalar]
                for b in range(B_LOC):
                    pt = pts[b]
                    for h in range(2):
                        ps_log = plog.tile([V, HT], f32, tag="pslog")
                        for c in range(6):
                            nc.tensor.matmul(ps_log, wt_sb[:, c, :],
                                             pt[:, c, h * HT:(h + 1) * HT],
                                             start=(c == 0), stop=(c == 5))
                        Es = E_sb[:, b, h, :]
                        nc.scalar.activation(Es, ps_log, AF.Exp, bias=bb_sb)
                        ps_z = pzp.tile([1, HT], f32, tag="psz")
                        nc.tensor.matmul(ps_z, ones_colV, Es,
                                         start=True, stop=True)
                        nc.vector.tensor_copy(ZS[:, b, h, :], ps_z)
                        gsel = g_sb if h == 0 else gb_sb
                        dstq = PQ if h == 0 else PQB
                        for j in range(NJ):
                            gidx = b * NJ + j
                            ps_g = pgatp.tile([128, HT], f32, tag="psg")
                            nc.tensor.matmul(ps_g, gsel[:, gidx, :],
                                             E_sb[:, b, h, :],
                                             start=True, stop=True)
                            dst = dstq[:, gidx, :]
                            if j % 3 == 2:
                                nc.vector.tensor_copy(dst, ps_g)
                            else:
                                nc.scalar.copy(dst, ps_g)
                predp_cm.__exit__(None, None, None)

                # lnZ correction (only gates final readout; overlaps DP)
                nc.scalar.activation(ZL.rearrange("p b h t -> p (b h t)"),
                                     ZS.rearrange("p b h t -> p (b h t)"),
                                     AF.Ln)
                nc.gpsimd.tensor_mul(ZM, ZL, ilm_sb)
                for b in range(B_LOC):
                    nc.scalar.activation(
                        ZL[:, b, :, :].rearrange("p h t -> p (h t)"),
                        ZM[:, b, :, :].rearrange("p h t -> p (h t)"),
                        AF.Copy, accum_out=SZ[:, b:b + 1])

            # ---------------- DP: interleaved fwd + bwd chains --------------
            with (
                tc.tile_pool(name="ppf", bufs=2, space="PSUM") as ppf,
                tc.tile_pool(name="ppb", bufs=2, space="PSUM") as ppb,
                tc.tile_pool(name="prs", bufs=1, space="PSUM") as prs,
                tc.tile_pool(name="pbr", bufs=1, space="PSUM") as pbr,
                tc.tile_pool(name="phalo", bufs=2, space="PSUM") as php,
            ):
                fslot = [0]
                bslot = [3]

                def rescale(state, pool, is_bwd):
                    ps36 = prs.tile([1, NCOL], f32, tag="ps36")
                    nc.tensor.matmul(ps36, cm_sb, state, start=True, stop=True)
                    s4 = smp.tile([1, B_LOC], f32, tag="s4")
                    nc.vector.tensor_reduce(
                        s4, ps36.rearrange("p (b j) -> p b j", j=NJ),
                        axis=AX.X, op=ALU.add)
                    if is_bwd:
                        k = bslot[0]; bslot[0] += 1
                        sm = smp.tile([1, B_LOC], f32, tag="sm")
                        nc.vector.tensor_mul(sm, s4, mrb_sb[:, :, k - 3])
                        nc.vector.tensor_add(SLOG[:, :, k], sm,
                                             mrb1_sb[:, :, k - 3])
                        s4e = smp.tile([1, B_LOC], f32, tag="s4e")
                        nc.vector.tensor_scalar_add(s4e, s4, EPS)
                        s4 = s4e
                    else:
                        k = fslot[0]; fslot[0] += 1
                        nc.scalar.copy(SLOG[:, :, k], s4)
                    rz4 = smp.tile([1, B_LOC], f32, tag="rz4")
                    nc.vector.reciprocal(rz4, s4)
                    psbr = pbr.tile([128, B_LOC], f32, tag="psbr")
                    nc.tensor.matmul(psbr, ones_row128f, rz4,
                                     start=True, stop=True)
                    out = pool.tile([128, NCOL], bf16,
                                    tag="Y" if is_bwd else "X")
                    for b in range(B_LOC):
                        nc.vector.tensor_scalar_mul(
                            out[:, b * NJ:(b + 1) * NJ],
                            state[:, b * NJ:(b + 1) * NJ], psbr[:, b:b + 1])
                    return out

                # init fwd X; init bwd round 0: Y = inj0 * PQB[499] (t=999)
                X = xp.tile([128, NCOL], bf16, tag="X")
                nc.vector.tensor_mul(X, PQ[:, :, 0], oh01_sb)
                Y = yp.tile([128, NCOL], bf16, tag="Y")
                nc.vector.tensor_mul(Y, inj0_sb, PQB[:, :, HT - 1])
                psb = ppb.tile([128, NCOL], f32, tag="psb")
                nc.tensor.matmul(psb, vb_sb, cb_sb[:, 0, :],
                                 start=True, stop=False)
                nc.tensor.matmul(psb, w1_sb, Y, start=False, stop=True)

                for k in range(1, HT):
                    # ---- bwd step t = 999 - k (k = 1..499 -> t=998..500) ----
                    # injection matmul first (start=True) so it runs OFF the
                    # bwd serial loop; the W1 matmul accumulates into it.
                    inj = k < NINJ
                    psb_n = ppb.tile([128, NCOL], f32, tag="psb")
                    if inj:
                        nc.tensor.matmul(psb_n, vb_sb, cb_sb[:, k, :],
                                         start=True, stop=False)
                    Y = yp.tile([128, NCOL], bf16, tag="Y")
                    nc.vector.tensor_mul(Y, psb, PQB[:, :, HT - 1 - k])
                    if k % REFRESH == 4:
                        yv = Y.rearrange("p (b j) -> p b j", j=NJ)
                        psh2 = php.tile([80, 32], f32, tag="psh")
                        nc.tensor.matmul(psh2, pr_sb, yv[:, :, 1:9],
                                         start=True, stop=True)
                        ph2v = psh2.rearrange("p (b j) -> p b j", j=8)
                        nc.vector.tensor_copy(yv[0:16, :, 0:8], ph2v[0:16])
                        nc.vector.tensor_copy(yv[64:80, :, 0:8], ph2v[64:80])
                    if k % RESC == 64:
                        Y = rescale(Y, yp, True)
                    nc.tensor.matmul(psb_n, w1_sb, Y,
                                     start=not inj, stop=True)
                    psb = psb_n

                    # ---- fwd step t = k (k = 1..499) ----
                    if k % RESC == 0:
                        X = rescale(X, xp, False)
                    psf = ppf.tile([128, NCOL], f32, tag="psf")
                    nc.tensor.matmul(psf, w1_sb, X, start=True, stop=True)
                    Xn = xp.tile([128, NCOL], bf16, tag="X")
                    nc.vector.tensor_mul(Xn, psf, PQ[:, :, k])
                    if k % REFRESH == 0:
                        xv = Xn.rearrange("p (b j) -> p b j", j=NJ)
                        psh = php.tile([80, 32], f32, tag="psh")
                        nc.tensor.matmul(psh, pr_sb, xv[:, :, 0:8],
                                         start=True, stop=True)
                        phv = psh.rearrange("p (b j) -> p b j", j=8)
                        nc.vector.tensor_copy(xv[0:16, :, 1:9], phv[0:16])
                        nc.vector.tensor_copy(xv[64:80, :, 1:9], phv[64:80])
                    X = Xn

                # ---------------- meet + readout ----------------------------
                Gs = smp.tile([128, NCOL], bf16, tag="Gs")
                nc.vector.tensor_copy(Gs, psb)
                gf1 = ppf.tile([128, NCOL], f32, tag="psf")
                nc.tensor.matmul(gf1, rev1_sb, Gs, start=True, stop=True)
                gf2 = ppb.tile([128, NCOL], f32, tag="psb")
                nc.tensor.matmul(gf2, rev2_sb, Gs, start=True, stop=True)
                D1 = smp.tile([128, NCOL], bf16, tag="D1")
                nc.vector.tensor_mul(D1, X, gf1)
                D2 = smp.tile([128, B_LOC, 8], bf16, tag="D2")
                xvw = X.rearrange("p (b j) -> p b j", j=NJ)
                gf2v = gf2.rearrange("p (b j) -> p b j", j=NJ)
                nc.vector.tensor_mul(D2, xvw[:, :, 0:8], gf2v[:, :, 1:9])
                ps1 = prs.tile([1, NCOL], f32, tag="ps36")
                nc.tensor.matmul(ps1, ones128, D1, start=True, stop=False,
                                 skip_group_check=True)
                p1v = ps1.rearrange("p (b j) -> p b j", j=NJ)
                nc.tensor.matmul(p1v[:, :, 0:8], ones128, D2,
                                 start=False, stop=True,
                                 skip_group_check=True)
                nc.vector.tensor_reduce(
                    SLOG[:, :, NSLOT - 1], p1v, axis=AX.X, op=ALU.add)
                logs = smp.tile([1, B_LOC, NSLOT], f32, tag="logs")
                nc.scalar.activation(logs.rearrange("p b k -> p (b k)"),
                                     SLOG.rearrange("p b k -> p (b k)"),
                                     AF.Ln)
                tot = smp.tile([1, B_LOC], f32, tag="tot")
                nc.vector.tensor_reduce(tot, logs, axis=AX.X, op=ALU.add)
                t1 = smp.tile([1, B_LOC], f32, tag="t1")
                nc.vector.tensor_sub(t1, c1il_sb, tot)
                nll = smp.tile([1, B_LOC], f32, tag="nll")
                nc.vector.tensor_add(nll, t1, SZ)
                yv_ = smp.tile([1, B_LOC], f32, tag="yv")
                nc.vector.tensor_mul(yv_, nll, rtl_sb)
                dma(y_d.rearrange("b one -> one b"), yv_)

    nc.compile()
    return nc


def build_in_maps(inputs):
    """Shard inputs + host-built constants -> one in_map per core."""
    bf = ml_dtypes.bfloat16
    pred = np.asarray(inputs["pred"], np.float32)
    targets = np.asarray(inputs["targets"]).astype(np.int64)
    in_len = np.asarray(inputs["input_lengths"]).astype(np.int64)
    tgt_len = np.asarray(inputs["target_lengths"]).astype(np.int64)
    Wm = np.asarray(inputs["W"], np.float32)
    bv = np.asarray(inputs["b"], np.float32)
    tgt2d = targets.reshape(B, L)
    predt_all = np.ascontiguousarray(pred.transpose(0, 2, 1)).astype(bf)
    wt = np.ascontiguousarray(Wm.T).astype(bf)
    bb = np.ascontiguousarray(bv.reshape(V, 1))
    in_maps = []
    for core in range(8):
        b0 = core * B_LOC
        cst = _build_core_consts(tgt2d, in_len, tgt_len, b0)
        im = dict(predt=np.ascontiguousarray(predt_all[b0:b0 + B_LOC]),
                  wt=wt, bb=bb)
        for k, v in cst.items():
            im[k] = np.ascontiguousarray(v)
        in_maps.append(im)
    return in_maps


_CACHED = {}


def kernel(**inputs):
    from concourse import bass_utils
    if "nc" not in _CACHED:
        _CACHED["nc"] = build_program()
    nc = _CACHED["nc"]
    in_maps = build_in_maps(inputs)
    res = bass_utils.run_bass_kernel_spmd(nc, in_maps, core_ids=list(range(8)))
    ys = [r["y"] for r in res.results]
    loss = np.concatenate([y.ravel() for y in ys]).astype(np.float64).sum() / B
    return np.float32(loss)


# revision 21
# speedup vs baseline: 1.1585x; 1.0076x over previous
"""Trainium2 Bass kernel for nn_CTCConsistencyLoss_7310034338203.

Data-parallel over batch: 8 cores x 4 samples. Per core:

  bulk:  fused per (b,h) pipeline overlapping the pred DMA:
         logits = predT.T @ W (bf16) -> E = exp(logits+b) ->
         per-state probability tiles via one-hot gather matmuls into
         TIME-MAJOR tiles PQ/PQB [128, 500, 36] (contiguous DP reads):
           PQ[:, t', :]  ascending rows,  t = t'        (fwd)
           PQB[:, t', :] descending rows, t = 500 + t'  (bwd)
  DP:    linear-domain CTC forward split into TWO independent chains:
           fwd:  X   <- (W1 @ X) .* PQ[k],          k = 1..499
           bwd:  g_t <- W1 @ (g_{t+1} .* PQB[..]),  t = 999..500
         The bwd serial loop (mul -> matmul -> mul) is the critical
         path; the injection matmul (readout one-hot at t == il) is
         HOISTED off it: inj(k) runs start=True early, the W1 matmul
         accumulates start=False.  Halo refresh every 8 steps: PR
         permutation matmul + two VECTOR copies.  Rescale every 128
         steps (fwd k%128==0, bwd k%128==64), logs deferred to one
         final Ln over a [1,4,8] stash.
  meet:  loss_lin = <alpha_499, g_500> via two host-built row-reversal
         matmuls + masked dot; nll = C1*il + ln(e1)*2tl - sum(logs)
         - ln(dot), y = nll/tl.
"""
import numpy as np
import ml_dtypes

B, T, D, V, L = 32, 1000, 768, 31, 200
S = 2 * L + 1
NJ = 9
CORE = 48
HALO = 16
BLK = 64
B_LOC = 4
NCOL = B_LOC * NJ            # 36
C1 = 2.5
E1 = 0.5                     # bf16-exact; E1^2 = 0.25 also bf16-exact.
                             # Tilt tuned so per-step lattice drift ~ 0:
                             # 128-step window sums stay well inside the
                             # Act-engine Ln table domain [2^-64, 2^64].
CTILT = float(np.log(E1))
RESC = 128
REFRESH = 8
NINJ = 200                   # injection window t in [800, 999]
NSLOT = 8                    # 3 fwd rescales + 4 bwd rescales + 1 dot
NRB = 4                      # bwd rescale count
EPS = 1e-30
HT = 500                     # half horizon


def _desc_pos(s):
    """Descending-layout position of state s: (col, row)."""
    jb = (s + HALO) // CORE
    return jb, CORE * jb + 47 - s


def _build_core_consts(tgt2d, in_len, tgt_len, b_base):
    """Host-built constants for one core (samples b_base..b_base+3)."""
    bf = ml_dtypes.bfloat16
    il = in_len[b_base:b_base + B_LOC].astype(np.int64)
    tl = tgt_len[b_base:b_base + B_LOC].astype(np.int64)
    ext = np.zeros((B_LOC, S), np.int64)
    ext[:, 1::2] = tgt2d[b_base:b_base + B_LOC]
    skip = np.zeros((B_LOC, S), bool)
    skip[:, 2:] = (ext[:, 2:] != 0) & (ext[:, 2:] != ext[:, :-2])
    m2s = np.zeros((B_LOC, S + 2), bool)
    m2s[:, :S - 2] = skip[:, 2:]

    # gather one-hots carry the per-frame boost kappa = E1 (bf16-exact);
    # PQ = kappa * E stays unnormalized -- the 1/Z and kappa corrections
    # are reclaimed at readout via sum(lnZ[t<il]) and CTILT*il.
    # ascending (fwd): masked rows use m2s (dest-skip)
    G = np.zeros((V, NCOL, 128), np.float32)
    # descending (bwd): masked rows use skip (source-skip)
    GB = np.zeros((V, NCOL, 128), np.float32)
    for b in range(B_LOC):
        for j in range(NJ):
            for m in range(BLK):
                s = CORE * j - HALO + m
                if 0 <= s <= 2 * int(tl[b]):
                    G[ext[b, s], b * NJ + j, m] = E1
                    if m2s[b, s]:
                        G[ext[b, s], b * NJ + j, 64 + m] = E1
                sd = CORE * j + 47 - m
                if 0 <= sd <= 2 * int(tl[b]):
                    GB[ext[b, sd], b * NJ + j, m] = E1
                    if skip[b, sd]:
                        GB[ext[b, sd], b * NJ + j, 64 + m] = E1

    e1, e2 = E1, E1 * E1
    W1 = np.zeros((128, 128), np.float32)
    for m in range(BLK):
        W1[m, m] = 1.0
        if m >= 1:
            W1[m - 1, m] = e1
        if m >= 2:
            W1[64 + m - 2, m] = e2
        W1[:, 64 + m] = W1[:, m]

    OH01 = np.zeros((128, NCOL), np.float32)
    for b in range(B_LOC):
        c = b * NJ
        OH01[16, c] = 1.0
        OH01[17, c] = e1
        OH01[80, c] = 1.0
        OH01[81, c] = e1

    # halo permutation: rows {48+m, 112+m} -> psum rows {m, 64+m}, m<16
    PR = np.zeros((128, 80), np.float32)
    for m in range(16):
        PR[48 + m, m] = 1.0
        PR[112 + m, 64 + m] = 1.0

    # bwd injection at end states {2tl-1 (w=e1), 2tl (w=1)}, desc coords
    INJ0 = np.zeros((128, NCOL), np.float32)
    VB = np.zeros((2 * B_LOC, 128), np.float32)
    CB = np.zeros((2 * B_LOC, NINJ, NCOL), np.float32)
    for b in range(B_LOC):
        for i, send in enumerate([2 * int(tl[b]) - 1, 2 * int(tl[b])]):
            w = e1 if i == 0 else 1.0
            jb, m = _desc_pos(send)
            if int(il[b]) == 1000:
                INJ0[m, b * NJ + jb] += w
                INJ0[64 + m, b * NJ + jb] += w
            else:
                VB[2 * b + i, m] = w
                VB[2 * b + i, 64 + m] = w
                k = 999 - int(il[b])
                CB[2 * b + i, k, b * NJ + jb] = 1.0

    CM = np.zeros((128, 1), np.float32)
    CM[16:64, 0] = 1.0           # core rows for BOTH layouts

    # meet reversal: fwd row r=16+u (state 48j+u) <- desc row (same col
    # j for u<32: 47-u; col j+1 for u>=32: 95-u)
    REV1 = np.zeros((128, 128), np.float32)
    REV2 = np.zeros((128, 128), np.float32)
    for u in range(32):
        REV1[47 - u, 16 + u] = 1.0
    for u in range(32, 48):
        REV2[95 - u, 16 + u] = 1.0

    # bwd rescale masks per slot: block at round r acts on g_{tb+1},
    # tb = 999 - k_r, k_r = 64 + 128*r: mask = il > tb
    MRB = np.zeros((1, B_LOC, NRB), np.float32)
    MRB1 = np.zeros((1, B_LOC, NRB), np.float32)
    for r in range(NRB):
        tb = 999 - (64 + RESC * r)
        mask = (il > tb).astype(np.float32)
        MRB[0, :, r] = mask
        MRB1[0, :, r] = 1.0 - mask

    # per-sample mask t < il over the [b, h, t'] layout (for sum lnZ_t)
    ILM = np.zeros((1, B_LOC, 2, HT), np.float32)
    for b in range(B_LOC):
        for h in range(2):
            for tp in range(HT):
                if h * HT + tp < int(il[b]):
                    ILM[0, b, h, tp] = 1.0

    C1IL = (CTILT * (il + 2 * tl)).astype(np.float32).reshape(1, B_LOC)
    RTL = (1.0 / tl.astype(np.float64)).astype(np.float32).reshape(1, B_LOC)
    return dict(g=G.reshape(V, NCOL * 128).astype(bf),
                gb=GB.reshape(V, NCOL * 128).astype(bf),
                w1=W1.astype(bf), pr=PR.astype(bf),
                oh01=OH01.astype(bf), inj0=INJ0.astype(bf),
                vb=VB.astype(bf),
                cb=CB.reshape(2 * B_LOC, NINJ * NCOL).astype(bf),
                cm=CM.astype(bf),
                rev1=REV1.astype(bf), rev2=REV2.astype(bf),
                mrb=MRB.reshape(1, B_LOC * NRB),
                mrb1=MRB1.reshape(1, B_LOC * NRB),
                ilm=ILM.reshape(1, B_LOC * 2 * HT).astype(bf),
                c1il=C1IL, rtl=RTL)


def build_program():
    import concourse.bacc as bacc
    import concourse.tile as tile
    from concourse import mybir

    f32 = mybir.dt.float32
    bf16 = mybir.dt.bfloat16
    AF = mybir.ActivationFunctionType
    ALU = mybir.AluOpType
    AX = mybir.AxisListType

    nc = bacc.Bacc("TRN2", target_bir_lowering=False, debug=False,
                   enable_asserts=False)

    # host pre-swizzled: predt[b, p, c*T + t] = pred[b, t, c*128 + p] so the
    # SBUF tile DMA is one contiguous 12 KB descriptor per partition
    predt = nc.dram_tensor("predt", [B_LOC, 128, 6 * T], bf16,
                           kind="ExternalInput").ap()
    wt = nc.dram_tensor("wt", [128, 6 * V], bf16, kind="ExternalInput").ap()
    bb = nc.dram_tensor("bb", [V, 1], f32, kind="ExternalInput").ap()
    g_d = nc.dram_tensor("g", [V, NCOL * 128], bf16, kind="ExternalInput").ap()
    gb_d = nc.dram_tensor("gb", [V, NCOL * 128], bf16, kind="ExternalInput").ap()
    w1_d = nc.dram_tensor("w1", [128, 128], bf16, kind="ExternalInput").ap()
    pr_d = nc.dram_tensor("pr", [128, 80], bf16, kind="ExternalInput").ap()
    oh01_d = nc.dram_tensor("oh01", [128, NCOL], bf16, kind="ExternalInput").ap()
    inj0_d = nc.dram_tensor("inj0", [128, NCOL], bf16, kind="ExternalInput").ap()
    vb_d = nc.dram_tensor("vb", [2 * B_LOC, 128], bf16, kind="ExternalInput").ap()
    cb_d = nc.dram_tensor("cb", [2 * B_LOC, NINJ * NCOL], bf16, kind="ExternalInput").ap()
    cm_d = nc.dram_tensor("cm", [128, 1], bf16, kind="ExternalInput").ap()
    rev1_d = nc.dram_tensor("rev1", [128, 128], bf16, kind="ExternalInput").ap()
    rev2_d = nc.dram_tensor("rev2", [128, 128], bf16, kind="ExternalInput").ap()
    mrb_d = nc.dram_tensor("mrb", [1, B_LOC * NRB], f32, kind="ExternalInput").ap()
    mrb1_d = nc.dram_tensor("mrb1", [1, B_LOC * NRB], f32, kind="ExternalInput").ap()
    ilm_d = nc.dram_tensor("ilm", [1, B_LOC * 2 * HT], bf16, kind="ExternalInput").ap()
    c1il_d = nc.dram_tensor("c1il", [1, B_LOC], f32, kind="ExternalInput").ap()
    rtl_d = nc.dram_tensor("rtl", [1, B_LOC], f32, kind="ExternalInput").ap()
    y_d = nc.dram_tensor("y", [B_LOC, 1], f32, kind="ExternalOutput").ap()

    dma = nc.default_dma_engine.dma_start

    with tile.TileContext(nc) as tc:
        with (
            tc.tile_pool(name="consts", bufs=1) as cp,
            tc.tile_pool(name="persist", bufs=1) as pp,
            tc.tile_pool(name="xs", bufs=3) as xp,
            tc.tile_pool(name="ys", bufs=3) as yp,
            tc.tile_pool(name="small", bufs=4) as smp,
        ):
            # wt first on its own (otherwise idle) queue so the first
            # logits matmul is gated only by pred[b=0]'s arrival.
            # Only sync + gpsimd issue DMAs: the tile scheduler charges a
            # dma_start's full transfer time to the issuing engine, so any
            # queue with bulk compute (Act does EXP + gather copies) would
            # stall that compute behind the transfers.
            wt_sb = cp.tile([128, 6, V], bf16, tag="wt")
            bb_sb = cp.tile([V, 1], f32, tag="bb")
            g_sbf = cp.tile([V, NCOL * 128], bf16, tag="g")
            gb_sbf = cp.tile([V, NCOL * 128], bf16, tag="gb")

            predp_cm = tc.tile_pool(name="predp", bufs=4)
            predp = predp_cm.__enter__()
            pts = []
            for b in range(B_LOC):
                pt = predp.tile([128, 6, T], bf16, tag="pt")
                pts.append(pt)
            # sync ring: b0 first (gates the first logits), then g, b2.
            # gpsimd ring: small consts first, then b1, b3.
            nc.sync.dma_start(pts[0], predt[0].rearrange("p (c t) -> p c t",
                                                         t=T))
            nc.gpsimd.dma_start(wt_sb, wt.rearrange("p (c v) -> p c v", v=V))
            nc.gpsimd.dma_start(bb_sb, bb)
            nc.gpsimd.dma_start(gb_sbf, gb_d)
            nc.sync.dma_start(g_sbf, g_d)
            nc.gpsimd.dma_start(pts[1], predt[1].rearrange("p (c t) -> p c t",
                                                           t=T))
            nc.sync.dma_start(pts[2], predt[2].rearrange("p (c t) -> p c t",
                                                         t=T))
            nc.gpsimd.dma_start(pts[3], predt[3].rearrange("p (c t) -> p c t",
                                                           t=T))
            w1_sb = cp.tile([128, 128], bf16, tag="w1")
            pr_sb = cp.tile([128, 80], bf16, tag="pr")
            oh01_sb = cp.tile([128, NCOL], bf16, tag="oh01")
            inj0_sb = cp.tile([128, NCOL], bf16, tag="inj0")
            vb_sb = cp.tile([2 * B_LOC, 128], bf16, tag="vb")
            cb_sbf = cp.tile([2 * B_LOC, NINJ * NCOL], bf16, tag="cb")
            cm_sb = cp.tile([128, 1], bf16, tag="cm")
            rev1_sb = cp.tile([128, 128], bf16, tag="rev1")
            rev2_sb = cp.tile([128, 128], bf16, tag="rev2")
            mrb_sb = cp.tile([1, B_LOC, NRB], f32, tag="mrb")
            mrb1_sb = cp.tile([1, B_LOC, NRB], f32, tag="mrb1")
            ilm_sb = cp.tile([1, B_LOC, 2, HT], bf16, tag="ilm")
            c1il_sb = cp.tile([1, B_LOC], f32, tag="c1il")
            rtl_sb = cp.tile([1, B_LOC], f32, tag="rtl")

            nc.gpsimd.dma_start(w1_sb, w1_d)
            nc.gpsimd.dma_start(pr_sb, pr_d)
            nc.gpsimd.dma_start(oh01_sb, oh01_d)
            nc.gpsimd.dma_start(inj0_sb, inj0_d)
            nc.gpsimd.dma_start(vb_sb, vb_d)
            nc.gpsimd.dma_start(cb_sbf, cb_d)
            nc.gpsimd.dma_start(cm_sb, cm_d)
            nc.gpsimd.dma_start(rev1_sb, rev1_d)
            nc.gpsimd.dma_start(rev2_sb, rev2_d)
            nc.gpsimd.dma_start(mrb_sb, mrb_d.rearrange("p (b k) -> p b k",
                                                        k=NRB))
            nc.gpsimd.dma_start(mrb1_sb, mrb1_d.rearrange("p (b k) -> p b k",
                                                          k=NRB))
            nc.gpsimd.dma_start(ilm_sb, ilm_d.rearrange("p (b h t) -> p b h t",
                                                        h=2, t=HT))
            nc.gpsimd.dma_start(c1il_sb, c1il_d)
            nc.gpsimd.dma_start(rtl_sb, rtl_d)
            g_sb = g_sbf.rearrange("v (c m) -> v c m", m=128)
            gb_sb = gb_sbf.rearrange("v (c m) -> v c m", m=128)
            cb_sb = cb_sbf.rearrange("p (k c) -> p k c", c=NCOL)

            ones_colV = cp.tile([V, 1], bf16, tag="ones_colV")
            nc.vector.memset(ones_colV, 1.0)
            ones_row128f = cp.tile([1, 128], f32, tag="ones_row128f")
            nc.vector.memset(ones_row128f, 1.0)
            ones128 = cp.tile([128, 1], bf16, tag="ones128")
            nc.vector.memset(ones128, 1.0)

            E_sb = pp.tile([V, B_LOC, 2, HT], bf16, tag="E")
            ZS = pp.tile([1, B_LOC, 2, HT], bf16, tag="ZS")
            ZL = pp.tile([1, B_LOC, 2, HT], bf16, tag="ZL")
            ZM = pp.tile([1, B_LOC, 2, HT], bf16, tag="ZM")
            SZ = pp.tile([1, B_LOC], f32, tag="SZ")
            PQ = pp.tile([128, NCOL, HT], bf16, tag="PQ")
            PQB = pp.tile([128, NCOL, HT], bf16, tag="PQB")
            SLOG = pp.tile([1, B_LOC, NSLOT], f32, tag="SLOG")
            nc.vector.memset(SLOG, 1.0)

            # ---------------- bulk: fused logits -> E -> gathers ------------
            with (
                tc.tile_pool(name="plog", bufs=2, space="PSUM") as plog,
                tc.tile_pool(name="pz", bufs=2, space="PSUM") as pzp,
                tc.tile_pool(name="pgat", bufs=3, space="PSUM") as pgatp,
            ):
                cp_eng = [nc.scalar, nc.vector, nc.scalar]
                for b in range(B_LOC):
                    pt = pts[b]
                    for h in range(2):
                        ps_log = plog.tile([V, HT], f32, tag="pslog")
                        for c in range(6):
                            nc.tensor.matmul(ps_log, wt_sb[:, c, :],
                                             pt[:, c, h * HT:(h + 1) * HT],
                                             start=(c == 0), stop=(c == 5))
                        Es = E_sb[:, b, h, :]
                        nc.scalar.activation(Es, ps_log, AF.Exp, bias=bb_sb)
                        ps_z = pzp.tile([1, HT], f32, tag="psz")
                        nc.tensor.matmul(ps_z, ones_colV, Es,
                                         start=True, stop=True)
                        nc.vector.tensor_copy(ZS[:, b, h, :], ps_z)
                        gsel = g_sb if h == 0 else gb_sb
                        dstq = PQ if h == 0 else PQB
                        for j in range(NJ):
                            gidx = b * NJ + j
                            ps_g = pgatp.tile([128, HT], f32, tag="psg")
                            nc.tensor.matmul(ps_g, gsel[:, gidx, :],
                                             E_sb[:, b, h, :],
                                             start=True, stop=True)
                            dst = dstq[:, gidx, :]
                            if j % 3 == 2:
                                nc.vector.tensor_copy(dst, ps_g)
                            else:
                                nc.scalar.copy(dst, ps_g)
                predp_cm.__exit__(None, None, None)

                # lnZ correction (only gates final readout; overlaps DP)
                nc.scalar.activation(ZL.rearrange("p b h t -> p (b h t)"),
                                     ZS.rearrange("p b h t -> p (b h t)"),
                                     AF.Ln)
                nc.gpsimd.tensor_mul(ZM, ZL, ilm_sb)
                for b in range(B_LOC):
                    nc.scalar.activation(
                        ZL[:, b, :, :].rearrange("p h t -> p (h t)"),
                        ZM[:, b, :, :].rearrange("p h t -> p (h t)"),
                        AF.Copy, accum_out=SZ[:, b:b + 1])

            # ---------------- DP: interleaved fwd + bwd chains --------------
            with (
                tc.tile_pool(name="ppf", bufs=2, space="PSUM") as ppf,
                tc.tile_pool(name="ppb", bufs=2, space="PSUM") as ppb,
                tc.tile_pool(name="prs", bufs=1, space="PSUM") as prs,
                tc.tile_pool(name="pbr", bufs=1, space="PSUM") as pbr,
                tc.tile_pool(name="phalo", bufs=2, space="PSUM") as php,
            ):
                fslot = [0]
                bslot = [3]

                def rescale(state, pool, is_bwd):
                    ps36 = prs.tile([1, NCOL], f32, tag="ps36")
                    nc.tensor.matmul(ps36, cm_sb, state, start=True, stop=True)
                    s4 = smp.tile([1, B_LOC], f32, tag="s4")
                    nc.vector.tensor_reduce(
                        s4, ps36.rearrange("p (b j) -> p b j", j=NJ),
                        axis=AX.X, op=ALU.add)
                    if is_bwd:
                        k = bslot[0]; bslot[0] += 1
                        sm = smp.tile([1, B_LOC], f32, tag="sm")
                        nc.vector.tensor_mul(sm, s4, mrb_sb[:, :, k - 3])
                        nc.vector.tensor_add(SLOG[:, :, k], sm,
                                             mrb1_sb[:, :, k - 3])
                        s4e = smp.tile([1, B_LOC], f32, tag="s4e")
                        nc.vector.tensor_scalar_add(s4e, s4, EPS)
                        s4 = s4e
                    else:
                        k = fslot[0]; fslot[0] += 1
                        nc.scalar.copy(SLOG[:, :, k], s4)
                    rz4 = smp.tile([1, B_LOC], f32, tag="rz4")
                    nc.vector.reciprocal(rz4, s4)
                    psbr = pbr.tile([128, B_LOC], f32, tag="psbr")
                    nc.tensor.matmul(psbr, ones_row128f, rz4,
                                     start=True, stop=True)
                    out = pool.tile([128, NCOL], bf16,
                                    tag="Y" if is_bwd else "X")
                    for b in range(B_LOC):
                        nc.vector.tensor_scalar_mul(
                            out[:, b * NJ:(b + 1) * NJ],
                            state[:, b * NJ:(b + 1) * NJ], psbr[:, b:b + 1])
                    return out

                # init fwd X; init bwd round 0: Y = inj0 * PQB[499] (t=999)
                X = xp.tile([128, NCOL], bf16, tag="X")
                nc.vector.tensor_mul(X, PQ[:, :, 0], oh01_sb)
                Y = yp.tile([128, NCOL], bf16, tag="Y")
                nc.vector.tensor_mul(Y, inj0_sb, PQB[:, :, HT - 1])
                psb = ppb.tile([128, NCOL], f32, tag="psb")
                nc.tensor.matmul(psb, vb_sb, cb_sb[:, 0, :],
                                 start=True, stop=False)
                nc.tensor.matmul(psb, w1_sb, Y, start=False, stop=True)

                for k in range(1, HT):
                    # ---- bwd step t = 999 - k (k = 1..499 -> t=998..500) ----
                    # injection matmul first (start=True) so it runs OFF the
                    # bwd serial loop; the W1 matmul accumulates into it.
                    inj = k < NINJ
                    psb_n = ppb.tile([128, NCOL], f32, tag="psb")
                    if inj:
                        nc.tensor.matmul(psb_n, vb_sb, cb_sb[:, k, :],
                                         start=True, stop=False)
                    Y = yp.tile([128, NCOL], bf16, tag="Y")
                    nc.vector.tensor_mul(Y, psb, PQB[:, :, HT - 1 - k])
                    if k % REFRESH == 4:
                        yv = Y.rearrange("p (b j) -> p b j", j=NJ)
                        psh2 = php.tile([80, 32], f32, tag="psh")
                        nc.tensor.matmul(psh2, pr_sb, yv[:, :, 1:9],
                                         start=True, stop=True)
                        ph2v = psh2.rearrange("p (b j) -> p b j", j=8)
                        nc.vector.tensor_copy(yv[0:16, :, 0:8], ph2v[0:16])
                        nc.vector.tensor_copy(yv[64:80, :, 0:8], ph2v[64:80])
                    if k % RESC == 64:
                        Y = rescale(Y, yp, True)
                    nc.tensor.matmul(psb_n, w1_sb, Y,
                                     start=not inj, stop=True)
                    psb = psb_n

                    # ---- fwd step t = k (k = 1..499) ----
                    if k % RESC == 0:
                        X = rescale(X, xp, False)
                    psf = ppf.tile([128, NCOL], f32, tag="psf")
                    nc.tensor.matmul(psf, w1_sb, X, start=True, stop=True)
                    Xn = xp.tile([128, NCOL], bf16, tag="X")
                    nc.vector.tensor_mul(Xn, psf, PQ[:, :, k])
                    if k % REFRESH == 0:
                        xv = Xn.rearrange("p (b j) -> p b j", j=NJ)
                        psh = php.tile([80, 32], f32, tag="psh")
                        nc.tensor.matmul(psh, pr_sb, xv[:, :, 0:8],
                                         start=True, stop=True)
                        phv = psh.rearrange("p (b j) -> p b j", j=8)
                        nc.vector.tensor_copy(xv[0:16, :, 1:9], phv[0:16])
                        nc.vector.tensor_copy(xv[64:80, :, 1:9], phv[64:80])
                    X = Xn

                # ---------------- meet + readout ----------------------------
                Gs = smp.tile([128, NCOL], bf16, tag="Gs")
                nc.vector.tensor_copy(Gs, psb)
                gf1 = ppf.tile([128, NCOL], f32, tag="psf")
                nc.tensor.matmul(gf1, rev1_sb, Gs, start=True, stop=True)
                gf2 = ppb.tile([128, NCOL], f32, tag="psb")
                nc.tensor.matmul(gf2, rev2_sb, Gs, start=True, stop=True)
                D1 = smp.tile([128, NCOL], bf16, tag="D1")
                nc.vector.tensor_mul(D1, X, gf1)
                D2 = smp.tile([128, B_LOC, 8], bf16, tag="D2")
                xvw = X.rearrange("p (b j) -> p b j", j=NJ)
                gf2v = gf2.rearrange("p (b j) -> p b j", j=NJ)
                nc.vector.tensor_mul(D2, xvw[:, :, 0:8], gf2v[:, :, 1:9])
                ps1 = prs.tile([1, NCOL], f32, tag="ps36")
                nc.tensor.matmul(ps1, ones128, D1, start=True, stop=False,
                                 skip_group_check=True)
                p1v = ps1.rearrange("p (b j) -> p b j", j=NJ)
                nc.tensor.matmul(p1v[:, :, 0:8], ones128, D2,
                                 start=False, stop=True,
                                 skip_group_check=True)
                nc.vector.tensor_reduce(
                    SLOG[:, :, NSLOT - 1], p1v, axis=AX.X, op=ALU.add)
                logs = smp.tile([1, B_LOC, NSLOT], f32, tag="logs")
                nc.scalar.activation(logs.rearrange("p b k -> p (b k)"),
                                     SLOG.rearrange("p b k -> p (b k)"),
                                     AF.Ln)
                tot = smp.tile([1, B_LOC], f32, tag="tot")
                nc.vector.tensor_reduce(tot, logs, axis=AX.X, op=ALU.add)
                t1 = smp.tile([1, B_LOC], f32, tag="t1")
                nc.vector.tensor_sub(t1, c1il_sb, tot)
                nll = smp.tile([1, B_LOC], f32, tag="nll")
                nc.vector.tensor_add(nll, t1, SZ)
                yv_ = smp.tile([1, B_LOC], f32, tag="yv")
                nc.vector.tensor_mul(yv_, nll, rtl_sb)
                dma(y_d.rearrange("b one -> one b"), yv_)

    nc.compile()
    return nc


def build_in_maps(inputs):
    """Shard inputs + host-built constants -> one in_map per core."""
    bf = ml_dtypes.bfloat16
    pred = np.asarray(inputs["pred"], np.float32)
    targets = np.asarray(inputs["targets"]).astype(np.int64)
    in_len = np.asarray(inputs["input_lengths"]).astype(np.int64)
    tgt_len = np.asarray(inputs["target_lengths"]).astype(np.int64)
    Wm = np.asarray(inputs["W"], np.float32)
    bv = np.asarray(inputs["b"], np.float32)
    tgt2d = targets.reshape(B, L)
    # [B, D, T] -> [B, 128, 6*T]: partition-major swizzle for contiguous DMA
    predt_all = np.ascontiguousarray(
        pred.transpose(0, 2, 1).reshape(B, 6, 128, T).transpose(0, 2, 1, 3)
        .reshape(B, 128, 6 * T)).astype(bf)
    wt = np.ascontiguousarray(
        Wm.T.reshape(6, 128, V).transpose(1, 0, 2).reshape(128, 6 * V)
    ).astype(bf)
    bb = np.ascontiguousarray(bv.reshape(V, 1))
    in_maps = []
    for core in range(8):
        b0 = core * B_LOC
        cst = _build_core_consts(tgt2d, in_len, tgt_len, b0)
        im = dict(predt=np.ascontiguousarray(predt_all[b0:b0 + B_LOC]),
                  wt=wt, bb=bb)
        for k, v in cst.items():
            im[k] = np.ascontiguousarray(v)
        in_maps.append(im)
    return in_maps


_CACHED = {}


def kernel(**inputs):
    from concourse import bass_utils
    if "nc" not in _CACHED:
        _CACHED["nc"] = build_program()
    nc = _CACHED["nc"]
    in_maps = build_in_maps(inputs)
    res = bass_utils.run_bass_kernel_spmd(nc, in_maps, core_ids=list(range(8)))
    ys = [r["y"] for r in res.results]
    loss = np.concatenate([y.ravel() for y in ys]).astype(np.float64).sum() / B
    return np.float32(loss)


# revision 27
# speedup vs baseline: 1.1930x; 1.0297x over previous
"""Trainium2 Bass kernel for nn_CTCConsistencyLoss_7310034338203.

Data-parallel over batch: 8 cores x 4 samples. Per core:

  bulk:  fused per (b,h) pipeline overlapping the pred DMA:
         logits = predT.T @ W (bf16) -> E = exp(logits+b) ->
         per-state probability tiles via one-hot gather matmuls into
         TIME-MAJOR tiles PQ/PQB [128, 500, 36] (contiguous DP reads):
           PQ[:, t', :]  ascending rows,  t = t'        (fwd)
           PQB[:, t', :] descending rows, t = 500 + t'  (bwd)
  DP:    linear-domain CTC forward split into TWO independent chains:
           fwd:  X   <- (W1 @ X) .* PQ[k],          k = 1..499
           bwd:  g_t <- W1 @ (g_{t+1} .* PQB[..]),  t = 999..500
         The bwd serial loop (mul -> matmul -> mul) is the critical
         path; the injection matmul (readout one-hot at t == il) is
         HOISTED off it: inj(k) runs start=True early, the W1 matmul
         accumulates start=False.  Halo refresh every 8 steps: PR
         permutation matmul + two VECTOR copies.  Rescale every 128
         steps (fwd k%128==0, bwd k%128==64), logs deferred to one
         final Ln over a [1,4,8] stash.
  meet:  loss_lin = <alpha_499, g_500> via two host-built row-reversal
         matmuls + masked dot; nll = C1*il + ln(e1)*2tl - sum(logs)
         - ln(dot), y = nll/tl.
"""
import numpy as np
import ml_dtypes

B, T, D, V, L = 32, 1000, 768, 31, 200
S = 2 * L + 1
NJ = 9
CORE = 48
HALO = 16
BLK = 64
B_LOC = 4
NCOL = B_LOC * NJ            # 36
C1 = 2.5
E1 = 0.5                     # bf16-exact; E1^2 = 0.25 also bf16-exact.
                             # Tilt tuned so per-step lattice drift ~ 0:
                             # 128-step window sums stay well inside the
                             # Act-engine Ln table domain [2^-64, 2^64].
CTILT = float(np.log(E1))
RESC = 128
REFRESH = 8
NINJ = 200                   # injection window t in [800, 999]
NSLOT = 8                    # 3 fwd rescales + 4 bwd rescales + 1 dot
NRB = 4                      # bwd rescale count
EPS = 1e-30
HT = 500                     # half horizon


def _desc_pos(s):
    """Descending-layout position of state s: (col, row)."""
    jb = (s + HALO) // CORE
    return jb, CORE * jb + 47 - s


def _build_core_consts(tgt2d, in_len, tgt_len, b_base):
    """Host-built constants for one core (samples b_base..b_base+3)."""
    bf = ml_dtypes.bfloat16
    il = in_len[b_base:b_base + B_LOC].astype(np.int64)
    tl = tgt_len[b_base:b_base + B_LOC].astype(np.int64)
    ext = np.zeros((B_LOC, S), np.int64)
    ext[:, 1::2] = tgt2d[b_base:b_base + B_LOC]
    skip = np.zeros((B_LOC, S), bool)
    skip[:, 2:] = (ext[:, 2:] != 0) & (ext[:, 2:] != ext[:, :-2])
    m2s = np.zeros((B_LOC, S + 2), bool)
    m2s[:, :S - 2] = skip[:, 2:]

    # gather one-hots carry the per-frame boost kappa = E1 (bf16-exact);
    # PQ = kappa * E stays unnormalized -- the 1/Z and kappa corrections
    # are reclaimed at readout via sum(lnZ[t<il]) and CTILT*il.
    # ascending (fwd): masked rows use m2s (dest-skip)
    G = np.zeros((V, NCOL, 128), np.float32)
    # descending (bwd): masked rows use skip (source-skip)
    GB = np.zeros((V, NCOL, 128), np.float32)
    for b in range(B_LOC):
        for j in range(NJ):
            for m in range(BLK):
                s = CORE * j - HALO + m
                if 0 <= s <= 2 * int(tl[b]):
                    G[ext[b, s], b * NJ + j, m] = E1
                    if m2s[b, s]:
                        G[ext[b, s], b * NJ + j, 64 + m] = E1
                sd = CORE * j + 47 - m
                if 0 <= sd <= 2 * int(tl[b]):
                    GB[ext[b, sd], b * NJ + j, m] = E1
                    if skip[b, sd]:
                        GB[ext[b, sd], b * NJ + j, 64 + m] = E1

    e1, e2 = E1, E1 * E1
    W1 = np.zeros((128, 128), np.float32)
    for m in range(BLK):
        W1[m, m] = 1.0
        if m >= 1:
            W1[m - 1, m] = e1
        if m >= 2:
            W1[64 + m - 2, m] = e2
        W1[:, 64 + m] = W1[:, m]

    OH01 = np.zeros((128, NCOL), np.float32)
    for b in range(B_LOC):
        c = b * NJ
        OH01[16, c] = 1.0
        OH01[17, c] = e1
        OH01[80, c] = 1.0
        OH01[81, c] = e1

    # halo permutation: rows {48+m, 112+m} -> psum rows {m, 64+m}, m<16
    PR = np.zeros((128, 80), np.float32)
    for m in range(16):
        PR[48 + m, m] = 1.0
        PR[112 + m, 64 + m] = 1.0

    # bwd injection at end states {2tl-1 (w=e1), 2tl (w=1)}, desc coords
    INJ0 = np.zeros((128, NCOL), np.float32)
    VB = np.zeros((2 * B_LOC, 128), np.float32)
    CB = np.zeros((2 * B_LOC, NINJ, NCOL), np.float32)
    for b in range(B_LOC):
        for i, send in enumerate([2 * int(tl[b]) - 1, 2 * int(tl[b])]):
            w = e1 if i == 0 else 1.0
            jb, m = _desc_pos(send)
            if int(il[b]) == 1000:
                INJ0[m, b * NJ + jb] += w
                INJ0[64 + m, b * NJ + jb] += w
            else:
                VB[2 * b + i, m] = w
                VB[2 * b + i, 64 + m] = w
                k = 999 - int(il[b])
                CB[2 * b + i, k, b * NJ + jb] = 1.0

    CM = np.zeros((128, 1), np.float32)
    CM[16:64, 0] = 1.0           # core rows for BOTH layouts

    # meet reversal: fwd row r=16+u (state 48j+u) <- desc row (same col
    # j for u<32: 47-u; col j+1 for u>=32: 95-u)
    REV1 = np.zeros((128, 128), np.float32)
    REV2 = np.zeros((128, 128), np.float32)
    for u in range(32):
        REV1[47 - u, 16 + u] = 1.0
    for u in range(32, 48):
        REV2[95 - u, 16 + u] = 1.0

    # bwd rescale masks per slot: block at round r acts on g_{tb+1},
    # tb = 999 - k_r, k_r = 64 + 128*r: mask = il > tb
    MRB = np.zeros((1, B_LOC, NRB), np.float32)
    MRB1 = np.zeros((1, B_LOC, NRB), np.float32)
    for r in range(NRB):
        tb = 999 - (64 + RESC * r)
        mask = (il > tb).astype(np.float32)
        MRB[0, :, r] = mask
        MRB1[0, :, r] = 1.0 - mask

    # per-sample mask t < il over the [b, h, t'] layout (for sum lnZ_t)
    ILM = np.zeros((1, B_LOC, 2, HT), np.float32)
    for b in range(B_LOC):
        for h in range(2):
            for tp in range(HT):
                if h * HT + tp < int(il[b]):
                    ILM[0, b, h, tp] = 1.0

    C1IL = (CTILT * (il + 2 * tl)).astype(np.float32).reshape(1, B_LOC)
    RTL = (1.0 / tl.astype(np.float64)).astype(np.float32).reshape(1, B_LOC)
    return dict(g=G.reshape(V, NCOL * 128).astype(bf),
                gb=GB.reshape(V, NCOL * 128).astype(bf),
                w1=W1.astype(bf), pr=PR.astype(bf),
                oh01=OH01.astype(bf), inj0=INJ0.astype(bf),
                vb=VB.astype(bf),
                cb=CB.reshape(2 * B_LOC, NINJ * NCOL).astype(bf),
                cm=CM.astype(bf),
                rev1=REV1.astype(bf), rev2=REV2.astype(bf),
                mrb=MRB.reshape(1, B_LOC * NRB),
                mrb1=MRB1.reshape(1, B_LOC * NRB),
                ilm=ILM.reshape(1, B_LOC * 2 * HT).astype(bf),
                c1il=C1IL, rtl=RTL)


def build_program():
    import concourse.bacc as bacc
    import concourse.tile as tile
    from concourse import mybir

    f32 = mybir.dt.float32
    bf16 = mybir.dt.bfloat16
    AF = mybir.ActivationFunctionType
    ALU = mybir.AluOpType
    AX = mybir.AxisListType

    nc = bacc.Bacc("TRN2", target_bir_lowering=False, debug=False,
                   enable_asserts=False)

    # host pre-swizzled: predt[b, p, c*T + t] = pred[b, t, c*128 + p] so the
    # SBUF tile DMA is one contiguous 12 KB descriptor per partition
    predt = nc.dram_tensor("predt", [B_LOC, 128, 6 * T], bf16,
                           kind="ExternalInput").ap()
    wt = nc.dram_tensor("wt", [128, 6 * V], bf16, kind="ExternalInput").ap()
    bb = nc.dram_tensor("bb", [V, 1], f32, kind="ExternalInput").ap()
    g_d = nc.dram_tensor("g", [V, NCOL * 128], bf16, kind="ExternalInput").ap()
    gb_d = nc.dram_tensor("gb", [V, NCOL * 128], bf16, kind="ExternalInput").ap()
    w1_d = nc.dram_tensor("w1", [128, 128], bf16, kind="ExternalInput").ap()
    pr_d = nc.dram_tensor("pr", [128, 80], bf16, kind="ExternalInput").ap()
    oh01_d = nc.dram_tensor("oh01", [128, NCOL], bf16, kind="ExternalInput").ap()
    inj0_d = nc.dram_tensor("inj0", [128, NCOL], bf16, kind="ExternalInput").ap()
    vb_d = nc.dram_tensor("vb", [2 * B_LOC, 128], bf16, kind="ExternalInput").ap()
    cb_d = nc.dram_tensor("cb", [2 * B_LOC, NINJ * NCOL], bf16, kind="ExternalInput").ap()
    cm_d = nc.dram_tensor("cm", [128, 1], bf16, kind="ExternalInput").ap()
    rev1_d = nc.dram_tensor("rev1", [128, 128], bf16, kind="ExternalInput").ap()
    rev2_d = nc.dram_tensor("rev2", [128, 128], bf16, kind="ExternalInput").ap()
    mrb_d = nc.dram_tensor("mrb", [1, B_LOC * NRB], f32, kind="ExternalInput").ap()
    mrb1_d = nc.dram_tensor("mrb1", [1, B_LOC * NRB], f32, kind="ExternalInput").ap()
    ilm_d = nc.dram_tensor("ilm", [1, B_LOC * 2 * HT], bf16, kind="ExternalInput").ap()
    c1il_d = nc.dram_tensor("c1il", [1, B_LOC], f32, kind="ExternalInput").ap()
    rtl_d = nc.dram_tensor("rtl", [1, B_LOC], f32, kind="ExternalInput").ap()
    y_d = nc.dram_tensor("y", [B_LOC, 1], f32, kind="ExternalOutput").ap()

    dma = nc.default_dma_engine.dma_start

    with tile.TileContext(nc) as tc:
        with (
            tc.tile_pool(name="consts", bufs=1) as cp,
            tc.tile_pool(name="persist", bufs=1) as pp,
            tc.tile_pool(name="xs", bufs=3) as xp,
            tc.tile_pool(name="ys", bufs=3) as yp,
            tc.tile_pool(name="small", bufs=4) as smp,
        ):
            # wt first on its own (otherwise idle) queue so the first
            # logits matmul is gated only by pred[b=0]'s arrival.
            # Only sync + gpsimd issue DMAs: the tile scheduler charges a
            # dma_start's full transfer time to the issuing engine, so any
            # queue with bulk compute (Act does EXP + gather copies) would
            # stall that compute behind the transfers.
            wt_sb = cp.tile([128, 6, V], bf16, tag="wt")
            bb_sb = cp.tile([V, 1], f32, tag="bb")
            g_sbf = cp.tile([V, NCOL * 128], bf16, tag="g")
            gb_sbf = cp.tile([V, NCOL * 128], bf16, tag="gb")

            predp_cm = tc.tile_pool(name="predp", bufs=4)
            predp = predp_cm.__enter__()
            pts = []
            for b in range(B_LOC):
                pt = predp.tile([128, 6, T], bf16, tag="pt")
                pts.append(pt)
            # sync ring: b0 first (gates the first logits), then g, b2.
            # gpsimd ring: small consts first, then b1, b3.
            nc.sync.dma_start(pts[0], predt[0].rearrange("p (c t) -> p c t",
                                                         t=T))
            nc.gpsimd.dma_start(wt_sb, wt.rearrange("p (c v) -> p c v", v=V))
            nc.gpsimd.dma_start(bb_sb, bb)
            nc.gpsimd.dma_start(gb_sbf, gb_d)
            nc.sync.dma_start(g_sbf, g_d)
            nc.gpsimd.dma_start(pts[1], predt[1].rearrange("p (c t) -> p c t",
                                                           t=T))
            nc.sync.dma_start(pts[2], predt[2].rearrange("p (c t) -> p c t",
                                                         t=T))
            nc.gpsimd.dma_start(pts[3], predt[3].rearrange("p (c t) -> p c t",
                                                           t=T))
            w1_sb = cp.tile([128, 128], bf16, tag="w1")
            pr_sb = cp.tile([128, 80], bf16, tag="pr")
            oh01_sb = cp.tile([128, NCOL], bf16, tag="oh01")
            inj0_sb = cp.tile([128, NCOL], bf16, tag="inj0")
            vb_sb = cp.tile([2 * B_LOC, 128], bf16, tag="vb")
            cb_sbf = cp.tile([2 * B_LOC, NINJ * NCOL], bf16, tag="cb")
            cm_sb = cp.tile([128, 1], bf16, tag="cm")
            rev1_sb = cp.tile([128, 128], bf16, tag="rev1")
            rev2_sb = cp.tile([128, 128], bf16, tag="rev2")
            mrb_sb = cp.tile([1, B_LOC, NRB], f32, tag="mrb")
            mrb1_sb = cp.tile([1, B_LOC, NRB], f32, tag="mrb1")
            ilm_sb = cp.tile([1, B_LOC, 2, HT], bf16, tag="ilm")
            c1il_sb = cp.tile([1, B_LOC], f32, tag="c1il")
            rtl_sb = cp.tile([1, B_LOC], f32, tag="rtl")

            nc.gpsimd.dma_start(w1_sb, w1_d)
            nc.gpsimd.dma_start(pr_sb, pr_d)
            nc.gpsimd.dma_start(oh01_sb, oh01_d)
            nc.gpsimd.dma_start(inj0_sb, inj0_d)
            nc.gpsimd.dma_start(vb_sb, vb_d)
            nc.gpsimd.dma_start(cb_sbf, cb_d)
            nc.gpsimd.dma_start(cm_sb, cm_d)
            nc.gpsimd.dma_start(rev1_sb, rev1_d)
            nc.gpsimd.dma_start(rev2_sb, rev2_d)
            nc.gpsimd.dma_start(mrb_sb, mrb_d.rearrange("p (b k) -> p b k",
                                                        k=NRB))
            nc.gpsimd.dma_start(mrb1_sb, mrb1_d.rearrange("p (b k) -> p b k",
                                                          k=NRB))
            nc.gpsimd.dma_start(ilm_sb, ilm_d.rearrange("p (b h t) -> p b h t",
                                                        h=2, t=HT))
            nc.gpsimd.dma_start(c1il_sb, c1il_d)
            nc.gpsimd.dma_start(rtl_sb, rtl_d)
            g_sb = g_sbf.rearrange("v (c m) -> v c m", m=128)
            gb_sb = gb_sbf.rearrange("v (c m) -> v c m", m=128)
            cb_sb = cb_sbf.rearrange("p (k c) -> p k c", c=NCOL)

            ones_colV = cp.tile([V, 1], bf16, tag="ones_colV")
            nc.vector.memset(ones_colV, 1.0)
            ones_row128f = cp.tile([1, 128], f32, tag="ones_row128f")
            nc.vector.memset(ones_row128f, 1.0)
            ones128 = cp.tile([128, 1], bf16, tag="ones128")
            nc.vector.memset(ones128, 1.0)

            E_sb = pp.tile([V, B_LOC, 2, HT], bf16, tag="E")
            ZS = pp.tile([1, B_LOC, 2, HT], bf16, tag="ZS")
            ZL = pp.tile([1, B_LOC, 2, HT], bf16, tag="ZL")
            ZM = pp.tile([1, B_LOC, 2, HT], bf16, tag="ZM")
            SZ = pp.tile([1, B_LOC], f32, tag="SZ")
            PQ = pp.tile([128, NCOL, HT], bf16, tag="PQ")
            PQB = pp.tile([128, NCOL, HT], bf16, tag="PQB")
            SLOG = pp.tile([1, B_LOC, NSLOT], f32, tag="SLOG")
            nc.vector.memset(SLOG, 1.0)

            # ---------------- bulk: fused logits -> E -> gathers ------------
            with (
                tc.tile_pool(name="plog", bufs=2, space="PSUM") as plog,
                tc.tile_pool(name="pz", bufs=2, space="PSUM") as pzp,
                tc.tile_pool(name="pgat", bufs=2, space="PSUM") as pgatp,
            ):
                for b in range(B_LOC):
                    pt = pts[b]
                    for h in range(2):
                        ps_log = plog.tile([V, HT], f32, tag="pslog")
                        for c in range(6):
                            nc.tensor.matmul(ps_log, wt_sb[:, c, :],
                                             pt[:, c, h * HT:(h + 1) * HT],
                                             start=(c == 0), stop=(c == 5))
                        Es = E_sb[:, b, h, :]
                        nc.scalar.activation(Es, ps_log, AF.Exp, bias=bb_sb)
                        ps_z = pzp.tile([1, HT], f32, tag="psz")
                        nc.tensor.matmul(ps_z, ones_colV, Es,
                                         start=True, stop=True)
                        nc.vector.tensor_copy(ZS[:, b, h, :], ps_z)
                        gsel = g_sb if h == 0 else gb_sb
                        dstq = PQ if h == 0 else PQB
                        # gathers in pairs sharing a 2-bank PSUM tile so one
                        # copy drains both
                        for jj in range(5):
                            j0 = 2 * jj
                            gidx = b * NJ + j0
                            # 512-f32 pitch: each half exactly one PSUM bank
                            ps_g = pgatp.tile([128, 2, 512], f32, tag="psg")
                            npair = 1 if j0 == NJ - 1 else 2
                            for u in range(npair):
                                nc.tensor.matmul(ps_g[:, u, 0:HT],
                                                 gsel[:, gidx + u, :],
                                                 E_sb[:, b, h, :],
                                                 start=True, stop=True)
                            src = ps_g[:, 0:npair, 0:HT]
                            dst = dstq[:, gidx:gidx + npair, :]
                            if jj % 2 == 1:
                                nc.vector.tensor_copy(dst, src)
                            else:
                                nc.scalar.copy(dst, src)
                predp_cm.__exit__(None, None, None)

                # lnZ correction (only gates final readout; overlaps DP)
                nc.scalar.activation(ZL.rearrange("p b h t -> p (b h t)"),
                                     ZS.rearrange("p b h t -> p (b h t)"),
                                     AF.Ln)
                nc.gpsimd.tensor_mul(ZM, ZL, ilm_sb)
                for b in range(B_LOC):
                    nc.scalar.activation(
                        ZL[:, b, :, :].rearrange("p h t -> p (h t)"),
                        ZM[:, b, :, :].rearrange("p h t -> p (h t)"),
                        AF.Copy, accum_out=SZ[:, b:b + 1])

            # ---------------- DP: interleaved fwd + bwd chains --------------
            with (
                tc.tile_pool(name="ppf", bufs=2, space="PSUM") as ppf,
                tc.tile_pool(name="ppb", bufs=2, space="PSUM") as ppb,
                tc.tile_pool(name="prs", bufs=1, space="PSUM") as prs,
                tc.tile_pool(name="pbr", bufs=1, space="PSUM") as pbr,
                tc.tile_pool(name="phalo", bufs=2, space="PSUM") as php,
            ):
                fslot = [0]
                bslot = [3]

                def rescale(state, pool, is_bwd):
                    ps36 = prs.tile([1, NCOL], f32, tag="ps36")
                    nc.tensor.matmul(ps36, cm_sb, state, start=True, stop=True)
                    s4 = smp.tile([1, B_LOC], f32, tag="s4")
                    nc.vector.tensor_reduce(
                        s4, ps36.rearrange("p (b j) -> p b j", j=NJ),
                        axis=AX.X, op=ALU.add)
                    if is_bwd:
                        k = bslot[0]; bslot[0] += 1
                        sm = smp.tile([1, B_LOC], f32, tag="sm")
                        nc.vector.tensor_mul(sm, s4, mrb_sb[:, :, k - 3])
                        nc.vector.tensor_add(SLOG[:, :, k], sm,
                                             mrb1_sb[:, :, k - 3])
                        s4e = smp.tile([1, B_LOC], f32, tag="s4e")
                        nc.vector.tensor_scalar_add(s4e, s4, EPS)
                        s4 = s4e
                    else:
                        k = fslot[0]; fslot[0] += 1
                        nc.scalar.copy(SLOG[:, :, k], s4)
                    rz4 = smp.tile([1, B_LOC], f32, tag="rz4")
                    nc.vector.reciprocal(rz4, s4)
                    psbr = pbr.tile([128, B_LOC, 1], f32, tag="psbr")
                    nc.tensor.matmul(psbr[:, :, 0], ones_row128f, rz4,
                                     start=True, stop=True)
                    out = pool.tile([128, NCOL], bf16,
                                    tag="Y" if is_bwd else "X")
                    # one mul: psbr broadcasts over j (stride-0 free dim)
                    nc.vector.tensor_mul(
                        out.rearrange("p (b j) -> p b j", j=NJ),
                        state.rearrange("p (b j) -> p b j", j=NJ),
                        psbr.broadcast_to([128, B_LOC, NJ]))
                    return out

                # init fwd X; init bwd round 0: Y = inj0 * PQB[499] (t=999)
                X = xp.tile([128, NCOL], bf16, tag="X")
                nc.vector.tensor_mul(X, PQ[:, :, 0], oh01_sb)
                Y = yp.tile([128, NCOL], bf16, tag="Y")
                nc.vector.tensor_mul(Y, inj0_sb, PQB[:, :, HT - 1])
                psb = ppb.tile([128, NCOL], f32, tag="psb")
                nc.tensor.matmul(psb, vb_sb, cb_sb[:, 0, :],
                                 start=True, stop=False)
                nc.tensor.matmul(psb, w1_sb, Y, start=False, stop=True)

                for k in range(1, HT):
                    # ---- bwd step t = 999 - k (k = 1..499 -> t=998..500) ----
                    # injection matmul first (start=True) so it runs OFF the
                    # bwd serial loop; the W1 matmul accumulates into it.
                    inj = k < NINJ
                    psb_n = ppb.tile([128, NCOL], f32, tag="psb")
                    if inj:
                        nc.tensor.matmul(psb_n, vb_sb, cb_sb[:, k, :],
                                         start=True, stop=False)
                    Y = yp.tile([128, NCOL], bf16, tag="Y")
                    nc.vector.tensor_mul(Y, psb, PQB[:, :, HT - 1 - k])
                    if k % REFRESH == 4:
                        yv = Y.rearrange("p (b j) -> p b j", j=NJ)
                        psh2 = php.tile([80, 32], f32, tag="psh")
                        nc.tensor.matmul(psh2, pr_sb, yv[:, :, 1:9],
                                         start=True, stop=True)
                        ph2v = psh2.rearrange("p (b j) -> p b j", j=8)
                        nc.scalar.copy(yv[0:16, :, 0:8], ph2v[0:16])
                        nc.vector.tensor_copy(yv[64:80, :, 0:8], ph2v[64:80])
                    if k % RESC == 64:
                        Y = rescale(Y, yp, True)
                    nc.tensor.matmul(psb_n, w1_sb, Y,
                                     start=not inj, stop=True)
                    psb = psb_n

                    # ---- fwd step t = k (k = 1..499) ----
                    if k % RESC == 0:
                        X = rescale(X, xp, False)
                    psf = ppf.tile([128, NCOL], f32, tag="psf")
                    nc.tensor.matmul(psf, w1_sb, X, start=True, stop=True)
                    Xn = xp.tile([128, NCOL], bf16, tag="X")
                    nc.vector.tensor_mul(Xn, psf, PQ[:, :, k])
                    if k % REFRESH == 0:
                        xv = Xn.rearrange("p (b j) -> p b j", j=NJ)
                        psh = php.tile([80, 32], f32, tag="psh")
                        nc.tensor.matmul(psh, pr_sb, xv[:, :, 0:8],
                                         start=True, stop=True)
                        phv = psh.rearrange("p (b j) -> p b j", j=8)
                        nc.scalar.copy(xv[0:16, :, 1:9], phv[0:16])
                        nc.vector.tensor_copy(xv[64:80, :, 1:9], phv[64:80])
                    X = Xn

                # ---------------- meet + readout ----------------------------
                Gs = smp.tile([128, NCOL], bf16, tag="Gs")
                nc.vector.tensor_copy(Gs, psb)
                gf1 = ppf.tile([128, NCOL], f32, tag="psf")
                nc.tensor.matmul(gf1, rev1_sb, Gs, start=True, stop=True)
                gf2 = ppb.tile([128, NCOL], f32, tag="psb")
                nc.tensor.matmul(gf2, rev2_sb, Gs, start=True, stop=True)
                D1 = smp.tile([128, NCOL], bf16, tag="D1")
                nc.vector.tensor_mul(D1, X, gf1)
                D2 = smp.tile([128, B_LOC, 8], bf16, tag="D2")
                xvw = X.rearrange("p (b j) -> p b j", j=NJ)
                gf2v = gf2.rearrange("p (b j) -> p b j", j=NJ)
                nc.vector.tensor_mul(D2, xvw[:, :, 0:8], gf2v[:, :, 1:9])
                ps1 = prs.tile([1, NCOL], f32, tag="ps36")
                nc.tensor.matmul(ps1, ones128, D1, start=True, stop=False,
                                 skip_group_check=True)
                p1v = ps1.rearrange("p (b j) -> p b j", j=NJ)
                nc.tensor.matmul(p1v[:, :, 0:8], ones128, D2,
                                 start=False, stop=True,
                                 skip_group_check=True)
                nc.vector.tensor_reduce(
                    SLOG[:, :, NSLOT - 1], p1v, axis=AX.X, op=ALU.add)
                logs = smp.tile([1, B_LOC, NSLOT], f32, tag="logs")
                nc.scalar.activation(logs.rearrange("p b k -> p (b k)"),
                                     SLOG.rearrange("p b k -> p (b k)"),
                                     AF.Ln)
                tot = smp.tile([1, B_LOC], f32, tag="tot")
                nc.vector.tensor_reduce(tot, logs, axis=AX.X, op=ALU.add)
                t1 = smp.tile([1, B_LOC], f32, tag="t1")
                nc.vector.tensor_sub(t1, c1il_sb, tot)
                nll = smp.tile([1, B_LOC], f32, tag="nll")
                nc.vector.tensor_add(nll, t1, SZ)
                yv_ = smp.tile([1, B_LOC], f32, tag="yv")
                nc.vector.tensor_mul(yv_, nll, rtl_sb)
                dma(y_d.rearrange("b one -> one b"), yv_)

    nc.compile()
    return nc


def build_in_maps(inputs):
    """Shard inputs + host-built constants -> one in_map per core."""
    bf = ml_dtypes.bfloat16
    pred = np.asarray(inputs["pred"], np.float32)
    targets = np.asarray(inputs["targets"]).astype(np.int64)
    in_len = np.asarray(inputs["input_lengths"]).astype(np.int64)
    tgt_len = np.asarray(inputs["target_lengths"]).astype(np.int64)
    Wm = np.asarray(inputs["W"], np.float32)
    bv = np.asarray(inputs["b"], np.float32)
    tgt2d = targets.reshape(B, L)
    # [B, D, T] -> [B, 128, 6*T]: partition-major swizzle for contiguous DMA
    predt_all = np.ascontiguousarray(
        pred.transpose(0, 2, 1).reshape(B, 6, 128, T).transpose(0, 2, 1, 3)
        .reshape(B, 128, 6 * T)).astype(bf)
    wt = np.ascontiguousarray(
        Wm.T.reshape(6, 128, V).transpose(1, 0, 2).reshape(128, 6 * V)
    ).astype(bf)
    bb = np.ascontiguousarray(bv.reshape(V, 1))
    in_maps = []
    for core in range(8):
        b0 = core * B_LOC
        cst = _build_core_consts(tgt2d, in_len, tgt_len, b0)
        im = dict(predt=np.ascontiguousarray(predt_all[b0:b0 + B_LOC]),
                  wt=wt, bb=bb)
        for k, v in cst.items():
            im[k] = np.ascontiguousarray(v)
        in_maps.append(im)
    return in_maps


_CACHED = {}


def kernel(**inputs):
    from concourse import bass_utils
    if "nc" not in _CACHED:
        _CACHED["nc"] = build_program()
    nc = _CACHED["nc"]
    in_maps = build_in_maps(inputs)
    res = bass_utils.run_bass_kernel_spmd(nc, in_maps, core_ids=list(range(8)))
    ys = [r["y"] for r in res.results]
    loss = np.concatenate([y.ravel() for y in ys]).astype(np.float64).sum() / B
    return np.float32(loss)
